# revision 56
# baseline (speedup 1.0000x reference)
"""Megatron-style tensor-parallel causal attention (BitLinear qkv/o) on 8 TRN2 cores.

Sharding: each core owns 2 of 16 heads (qkv_weight rows) and the matching
256 o_weight columns. x/rotary replicated; partial outputs summed on host.

All SBUF data is f16 (halves DMA + enables DVE 2x modes); PSUM stays fp32.
Causal mask is folded into the score PSUM via an identity-lhsT matmul adding
-60 to masked entries before exp. Softmax denominator uses the all-ones
lhsT matmul (broadcast rows), normalization on DVE before the out-proj.
Emission order proj(b0), proj(b1), attn(b0), attn(b1) keeps the PE busy:
RoPE(b0) on DVE overlaps proj(b1) matmuls, attention overlaps nothing it
needs.
"""

import math

import numpy as np

EPS = 1e-5
NUM_HEADS = 16
HEAD_DIM = 128
B, S, H = 2, 2048, 2048
NCORES = 8
HPC = NUM_HEADS // NCORES        # heads per core = 2
FPC = 3 * HPC * HEAD_DIM         # qkv features per core = 768
P = 128
NHT = H // P                     # 16 h_in tiles
CH = 512                         # proj token chunk
NCH = S // CH                    # 4 chunks per batch
QC = 256                         # attention q chunk
NQC = S // QC                    # 8
MASKV = -60.0


def _build_program():
    import concourse.bacc as bacc
    import concourse.mybir as mybir
    import concourse.tile as tile

    f32 = mybir.dt.float32
    f16 = mybir.dt.float16
    AF = mybir.ActivationFunctionType

    nc = bacc.Bacc(None, target_bir_lowering=False)

    xt = nc.dram_tensor("xt", [B, H, S], f16, kind="ExternalInput")
    wqkv = nc.dram_tensor("wqkv", [H, FPC], f16, kind="ExternalInput")
    wo = nc.dram_tensor("wo", [HPC * HEAD_DIM, H], f16, kind="ExternalInput")
    cos_t = nc.dram_tensor("cos_t", [P, S], f16, kind="ExternalInput")
    sin_s = nc.dram_tensor("sin_s", [P, S], f16, kind="ExternalInput")
    # aux: [0:512) mask pair (B0|B1), [512:640) identity, [640:768) ones
    aux = nc.dram_tensor("aux", [P, 832], f16, kind="ExternalInput")
    out = nc.dram_tensor("out", [B, S, H], f16, kind="ExternalOutput")

    with tile.TileContext(nc) as tc:
        with tc.tile_pool(name="const", bufs=1) as cpool:
            # first proj chunk's x and the first weight slice lead the DMA
            # queue so the PE starts ~9us in instead of ~24us.
            w_sb = cpool.tile([P, NHT, FPC], f16)
            wre = wqkv.rearrange("(t p) f -> p t f", p=P)
            nc.sync.dma_start(w_sb[:, 0:4, :], wre[:, 0:4, :])

            with (
                tc.tile_pool(name="qk", bufs=2) as qkpool,
                tc.tile_pool(name="vv", bufs=2) as vpool,
                tc.tile_pool(name="work", bufs=2) as wpool,
                tc.tile_pool(name="attn", bufs=3) as apool,
                tc.tile_pool(name="outp", bufs=3) as opool,
                tc.psum_pool(name="pproj", bufs=2) as pps,
                tc.psum_pool(name="pop", bufs=2) as opps,
            ):
                # batch-0 leads with two small 256-token chunks so the first
                # proj matmuls start while weights are still streaming in
                chunks = {
                    0: [(0, 256), (256, 256), (512, 512), (1024, 512), (1536, 512)],
                    1: [(c * CH, CH) for c in range(NCH)],
                }
                # startup: interleave w and first-bite x by h-group
                xt0 = wpool.tile([P, NHT, 256], f16, tag="xt")
                xre0 = xt[0, :, 0:256].rearrange("(t p) c -> p t c", p=P)
                nc.sync.dma_start(xt0[:, 0:4, :], xre0[:, 0:4, :])
                for hgrp in range(1, 4):
                    nc.sync.dma_start(
                        w_sb[:, 4 * hgrp : 4 * (hgrp + 1), :],
                        wre[:, 4 * hgrp : 4 * (hgrp + 1), :],
                    )
                    nc.sync.dma_start(
                        xt0[:, 4 * hgrp : 4 * (hgrp + 1), :],
                        xre0[:, 4 * hgrp : 4 * (hgrp + 1), :],
                    )
                # second bite before the (later-needed) constants
                xt1 = wpool.tile([P, NHT, 256], f16, tag="xt")
                nc.sync.dma_start(
                    xt1[:], xt[0, :, 256:512].rearrange("(t p) c -> p t c", p=P)
                )
                wo_sb = cpool.tile([P, HPC, H], f16)
                nc.sync.dma_start(wo_sb[:], wo.rearrange("(t p) o -> p t o", p=P))
                aux_sb = cpool.tile([P, 832], f16)
                nc.sync.dma_start(aux_sb[:], aux[:])
                rot_sb = cpool.tile([P, 2 * S], f16)
                nc.sync.dma_start(rot_sb[:, 0:S], cos_t[:])
                nc.sync.dma_start(rot_sb[:, S : 2 * S], sin_s[:])

                msk = aux_sb[:, 0:512]          # [k,128] x (B0|B1) for diag pair
                iden = aux_sb[:, 512:640]       # identity
                ones = aux_sb[:, 640:768]       # all-ones
                expb = aux_sb[:, 768:769]       # exp bias column (-8)

                qk_raw = {}   # (b, f) -> raw (pre-rope) tiles
                qk_rope = {}  # (b, f) -> roped tiles
                v_sb = {}     # b -> v tiles [tok_part, ktile, hl*128]
                for b in range(B):
                    for f in range(4):
                        qk_raw[b, f] = qkpool.tile(
                            [P, S], f16, tag=f"qkr{f}", name=f"qkr{f}_{b}"
                        )
                        qk_rope[b, f] = qkpool.tile(
                            [P, S], f16, tag=f"qkf{f}", name=f"qkf{f}_{b}"
                        )
                    v_sb[b] = vpool.tile(
                        [P, (S // P) * 2 * P], f16, tag="v", name=f"v_{b}"
                    )

                # ---------------- projection (+rope) -----------------------
                xt_pre = {}

                def prefetch_xt(b, ci):
                    t0c, W = chunks[b][ci]
                    t = wpool.tile(
                        [P, NHT, W], f16, tag="xt", name=f"xt_{b}_{ci}"
                    )
                    nc.sync.dma_start(
                        t[:],
                        xt[b, :, t0c : t0c + W].rearrange("(t p) c -> p t c", p=P),
                    )
                    xt_pre[b, ci] = t

                def proj_chunk(b, ci):
                    t0c, W = chunks[b][ci]
                    if True:
                        if b == 0 and ci == 0:
                            xt_sb = xt0
                        elif b == 0 and ci == 1:
                            xt_sb = xt1
                        elif (b, ci) in xt_pre:
                            xt_sb = xt_pre.pop((b, ci))
                        else:
                            xt_sb = wpool.tile(
                                [P, NHT, W], f16,
                                tag="xt", name=f"xt_{b}_{ci}",
                            )
                            nc.sync.dma_start(
                                xt_sb[:],
                                xt[b, :, t0c : t0c + W].rearrange(
                                    "(t p) c -> p t c", p=P
                                ),
                            )
                        # q0,q1,k0,k1 : [feat, tok]
                        for f in range(4):
                            ps = pps.tile([P, W], f32, tag="proj", name=f"ps{b}_{ci}_{f}")
                            for h in range(NHT):
                                nc.tensor.matmul(
                                    ps[:],
                                    lhsT=w_sb[:, h, f * P : (f + 1) * P],
                                    rhs=xt_sb[:, h, :],
                                    start=(h == 0),
                                    stop=(h == NHT - 1),
                                )
                            if f % 2 == 0:
                                nc.scalar.copy(
                                    qk_raw[b, f][:, t0c : t0c + W], ps[:]
                                )
                            else:
                                nc.vector.tensor_copy(
                                    qk_raw[b, f][:, t0c : t0c + W], ps[:]
                                )
                        # v: [tok, feat] two tok-subs per psum tile
                        for half in range(W // 256):
                            psv = pps.tile(
                                [P, 512], f32, tag="proj", name=f"psv{b}_{ci}_{half}"
                            )
                            for sub in range(2):
                                tsub = half * 2 + sub
                                for h in range(NHT):
                                    nc.tensor.matmul(
                                        psv[:, sub * 2 * P : (sub + 1) * 2 * P],
                                        lhsT=xt_sb[:, h, tsub * P : (tsub + 1) * P],
                                        rhs=w_sb[:, h, 4 * P : 6 * P],
                                        start=(h == 0),
                                        stop=(h == NHT - 1),
                                    )
                            kt0 = t0c // P + half * 2
                            if half == 0:
                                nc.scalar.copy(
                                    v_sb[b][:, kt0 * 2 * P : (kt0 + 2) * 2 * P], psv[:]
                                )
                            else:
                                nc.vector.tensor_copy(
                                    v_sb[b][:, kt0 * 2 * P : (kt0 + 2) * 2 * P], psv[:]
                                )
                def rope_piece(b, pi):
                    # rope one proj chunk's span; runs on DVE under the next
                    # chunk's proj matmuls
                    t0c, W = chunks[b][pi]
                    for f in range(4):
                        raw = qk_raw[b, f]
                        qsw = wpool.tile(
                            [P, W], f16, tag="qsw", name=f"qsw{b}_{pi}_{f}"
                        )
                        nc.sync.dma_start(
                            qsw[0:64, :], raw[64:128, t0c : t0c + W]
                        )
                        nc.sync.dma_start(
                            qsw[64:128, :], raw[0:64, t0c : t0c + W]
                        )
                        m1 = wpool.tile(
                            [P, W], f16, tag="m1", name=f"m1{b}_{pi}_{f}"
                        )
                        nc.vector.tensor_mul(
                            m1[:], raw[:, t0c : t0c + W], rot_sb[:, t0c : t0c + W]
                        )
                        nc.vector.tensor_mul(
                            qsw[:], qsw[:], rot_sb[:, S + t0c : S + t0c + W]
                        )
                        nc.vector.tensor_add(
                            qk_rope[b, f][:, t0c : t0c + W], m1[:], qsw[:]
                        )

                # ---------------- attention + out-proj ----------------------
                # The last k-tile of each q-chunk only covers q[128:256)
                # (ragged trim). Denominator: full pairs are pre-summed on DVE
                # (halves the ones-matmul rows); the ones-matmul for pair g is
                # deferred until after pair g+1's attn*v so the PE never waits
                # on the DVE add.
                def attn_unit(b, qc, hl):
                    q_t = qk_rope[b, hl]
                    k_t = qk_rope[b, 2 + hl]
                    qs = q_t[:, qc * QC : (qc + 1) * QC]
                    qs_hi = q_t[:, qc * QC + P : (qc + 1) * QC]
                    yt = opps.tile([P, 512], f32, tag="op", name=f"yt{b}_{qc}_{hl}", bufs=5)
                    sm = opps.tile([P, QC], f32, tag="sum", name=f"sm{b}_{qc}_{hl}", bufs=1)
                    pend = None       # deferred exs tile for the ones-matmul
                    sum_started = False

                    def ones_mm(rhs_ap, region, stop):
                        nonlocal sum_started
                        nc.tensor.matmul(
                            sm[:, region[0] : region[1]],
                            lhsT=ones,
                            rhs=rhs_ap,
                            start=not sum_started,
                            stop=stop,
                        )
                        sum_started = True

                    def emit_scores(g):
                        diag = g == qc
                        sc = opps.tile(
                            [P, 2 * QC], f32, tag="op", bufs=5,
                            name=f"sc{b}_{qc}_{hl}_{g}",
                        )
                        nc.tensor.matmul(
                            sc[:, 0:QC],
                            lhsT=k_t[:, 2 * g * P : (2 * g + 1) * P],
                            rhs=qs,
                            start=True,
                            stop=not diag,
                        )
                        if diag:
                            nc.tensor.matmul(
                                sc[:, 0:QC], lhsT=iden, rhs=msk[:, 0:QC],
                                start=False, stop=True,
                            )
                            nc.tensor.matmul(
                                sc[:, QC : QC + P],
                                lhsT=k_t[:, (2 * g + 1) * P : (2 * g + 2) * P],
                                rhs=qs_hi,
                                start=True,
                                stop=False,
                            )
                            nc.tensor.matmul(
                                sc[:, QC : QC + P], lhsT=iden, rhs=msk[:, 0:P],
                                start=False, stop=True,
                            )
                        else:
                            nc.tensor.matmul(
                                sc[:, QC : 2 * QC],
                                lhsT=k_t[:, (2 * g + 1) * P : (2 * g + 2) * P],
                                rhs=qs,
                                start=True,
                                stop=True,
                            )
                        return sc

                    # 3-stage pipeline: scores(g+2) and exp(g+1) run ahead of
                    # attn*v(g), so the PE never waits on the Activation
                    # engine's exp. Denominator adds (DVE) get a full
                    # iteration of slack before their ones-matmul.
                    exd = {}   # g -> (ex tile, exs tile or None)

                    def emit_exp(g):
                        diag = g == qc
                        scw = 2 * QC if not diag else QC + P
                        ex = apool.tile([P, scw], f16, tag="ex")
                        nc.scalar.activation(
                            ex[:], scd[g][:, 0:scw], AF.Exp, bias=expb
                        )
                        exs = None
                        if not diag:
                            exs = apool.tile([P, QC], f16, tag="exs")
                            nc.vector.tensor_add(
                                exs[:], ex[:, 0:QC], ex[:, QC : 2 * QC]
                            )
                        exd[g] = (ex, exs)

                    def emit_av(g):
                        diag = g == qc
                        scw = 2 * QC if not diag else QC + P
                        ex, exs = exd.pop(g)
                        v0 = 2 * g * 2 * P + hl * P
                        nc.tensor.matmul(
                            yt[:, 0:QC],
                            lhsT=v_sb[b][:, v0 : v0 + P],
                            rhs=ex[:, 0:QC],
                            start=(g == 0),
                            stop=False,
                        )
                        v1 = (2 * g + 1) * 2 * P + hl * P
                        nc.tensor.matmul(
                            yt[:, P:QC] if diag else yt[:, 0:QC],
                            lhsT=v_sb[b][:, v1 : v1 + P],
                            rhs=ex[:, QC:scw],
                            start=False,
                            stop=diag,
                        )
                        if not diag:
                            ones_mm(exs[:], (0, QC), stop=False)
                        else:
                            ones_mm(ex[:, 0:QC], (0, QC), stop=False)
                            ones_mm(ex[:, QC : QC + P], (P, QC), stop=True)

                    scd = {0: emit_scores(0)}
                    if qc >= 1:
                        scd[1] = emit_scores(1)
                    for g in range(qc + 1):
                        emit_exp(g)
                        if g + 2 <= qc:
                            scd[g + 2] = emit_scores(g + 2)
                        if g >= 1:
                            emit_av(g - 1)
                    emit_av(qc)
                    recip = apool.tile([P, QC], f32, tag="rc")
                    nc.vector.reciprocal(recip[:], sm[:])
                    y = apool.tile([P, QC], f16, tag=f"yn{hl}")
                    nc.vector.tensor_mul(y[:], yt[:, 0:QC], recip[:])
                    return y

                def oproj_part(b, qc, yn, quarters, os_sb):
                    # sub-interleaved so each quarter's PSUM drain overlaps
                    # the other sub's matmuls
                    for quarter in quarters:
                        for sub in range(2):
                            ops = opps.tile([P, 512], f32, tag="op", bufs=5)
                            for hl in range(2):
                                nc.tensor.matmul(
                                    ops[:],
                                    lhsT=yn[hl][:, sub * P : (sub + 1) * P],
                                    rhs=wo_sb[:, hl, quarter * 512 : (quarter + 1) * 512],
                                    start=(hl == 0),
                                    stop=(hl == 1),
                                )
                            if (quarter * 2 + sub) % 2 == 0:
                                nc.scalar.copy(
                                    os_sb[sub][:, quarter * 512 : (quarter + 1) * 512],
                                    ops[:],
                                )
                            else:
                                nc.vector.tensor_copy(
                                    os_sb[sub][:, quarter * 512 : (quarter + 1) * 512],
                                    ops[:],
                                )

                def oproj_alloc(b, qc):
                    return [
                        opool.tile([P, H], f16, tag="os", name=f"os{b}_{qc}_{s}")
                        for s in range(2)
                    ]

                def oproj_flush(b, qc, os_sb):
                    for sub in range(2):
                        t0 = qc * QC + sub * P
                        nc.sync.dma_start(out[b, t0 : t0 + P, :], os_sb[sub][:])

                pending = None

                def attn_step(b, qc):
                    # previous chunk's out-proj lands in two half-bursts
                    # around this chunk's second head-unit: PE work that
                    # hides the normalize chain and spreads evacuations
                    nonlocal pending
                    y0 = attn_unit(b, qc, 0)
                    if pending is not None:
                        pb, pqc, pyn = pending
                        osb = oproj_alloc(pb, pqc)
                        oproj_part(pb, pqc, pyn, (0, 1), osb)
                    y1 = attn_unit(b, qc, 1)
                    if pending is not None:
                        oproj_part(pb, pqc, pyn, (2, 3), osb)
                        oproj_flush(pb, pqc, osb)
                    pending = (b, qc, [y0, y1])

                # fully pipelined schedule: every attention unit runs in a
                # projection window as soon as causally possible, so the
                # exp/evac load always has proj matmuls to hide under and
                # there is no attention-only tail
                b0_units = {0: [], 1: [(0, 0)], 2: [(0, 1), (0, 2), (0, 3)],
                            3: [(0, 4), (0, 5)], 4: [(0, 6)]}
                b1_units = {0: [(0, 7), (1, 0), (1, 1)], 1: [(1, 2), (1, 3)],
                            2: [(1, 4), (1, 5)], 3: [(1, 6), (1, 7)]}
                for pi in range(len(chunks[0])):
                    proj_chunk(0, pi)
                    if pi + 1 < len(chunks[0]):
                        if pi + 1 >= 2:
                            prefetch_xt(0, pi + 1)
                    else:
                        prefetch_xt(1, 0)
                    rope_piece(0, pi)
                    for ub, uqc in b0_units[pi]:
                        attn_step(ub, uqc)
                for ci in range(len(chunks[1])):
                    proj_chunk(1, ci)
                    if ci + 1 < len(chunks[1]):
                        prefetch_xt(1, ci + 1)
                    rope_piece(1, ci)
                    for ub, uqc in b1_units[ci]:
                        attn_step(ub, uqc)
                pb, pqc, pyn = pending
                osb = oproj_alloc(pb, pqc)
                oproj_part(pb, pqc, pyn, (0, 1), osb)
                oproj_part(pb, pqc, pyn, (2, 3), osb)
                oproj_flush(pb, pqc, osb)
    nc.finalize()
    return nc


_NC_CACHE = None


def _get_program():
    global _NC_CACHE
    if _NC_CACHE is None:
        _NC_CACHE = _build_program()
    return _NC_CACHE


def _prep_in_maps(x, rotary, qkv_weight, o_weight):
    import jax
    import ml_dtypes

    bf = np.float16
    cpu = jax.devices("cpu")[0]
    with jax.default_device(cpu):
        import jax.numpy as jnp

        sq = jnp.mean(jnp.abs(jnp.asarray(qkv_weight)))
        wq_q = np.asarray(jnp.round(jnp.asarray(qkv_weight) / (sq + EPS)), np.float32)
        so = jnp.mean(jnp.abs(jnp.asarray(o_weight)))
        wo_q = np.asarray(jnp.round(jnp.asarray(o_weight) / (so + EPS)), np.float32)
        sq = float(sq)
        so = float(so)

    xt = np.ascontiguousarray(x.transpose(0, 2, 1)).astype(bf)
    cos_t = np.ascontiguousarray(rotary[1].T).astype(bf)
    sin_t = np.ascontiguousarray(rotary[0].T).astype(np.float32)
    sin_s = sin_t.copy()
    sin_s[:64] *= -1.0
    sin_s = sin_s.astype(bf)

    # aux: mask pair for the diagonal k-tile pair, identity, ones
    kk = np.arange(P)[:, None]
    qq = np.arange(QC)[None, :]
    aux = np.zeros((P, 832), np.float32)
    aux[:, 0:QC] = np.where(qq < kk, MASKV, 0.0)          # B0: k-tile 2qc
    aux[:, QC : 2 * QC] = np.where(qq < kk + P, MASKV, 0.0)  # B1: k-tile 2qc+1
    aux[:, 512:640] = np.eye(P)
    aux[:, 640:768] = 1.0
    aux[:, 768] = -8.0
    aux = aux.astype(bf)

    # fp16 scaling: sqrt(sm_scale) on BOTH q and k weights (scores land fully
    # scaled in PSUM, masks are in post-scale units); final o-scale folded
    # into v weights (keeps every fp16 tensor in normal range; o_weight stays
    # exactly ternary in fp16).
    alpha = np.float32(math.sqrt(sq * sq / math.sqrt(HEAD_DIM)))
    final_scale = np.float32(sq * so)

    in_maps = []
    for c in range(NCORES):
        rows = []
        for part in range(3):  # q, k, v blocks of qkv_weight
            for hl in range(HPC):
                g = HPC * c + hl
                blk = wq_q[part * H + g * HEAD_DIM : part * H + (g + 1) * HEAD_DIM]
                if part < 2:
                    blk = blk * alpha
                else:
                    blk = blk * final_scale
                rows.append(blk)
        wqkv_c = np.ascontiguousarray(np.concatenate(rows, axis=0).T).astype(bf)
        wo_c = np.ascontiguousarray(
            wo_q[:, c * FPC // 3 : (c + 1) * FPC // 3].T
        ).astype(bf)
        in_maps.append(
            {
                "xt": xt,
                "wqkv": wqkv_c,
                "wo": wo_c,
                "cos_t": cos_t,
                "sin_s": sin_s,
                "aux": aux,
            }
        )
    return in_maps


def kernel(x, rotary, qkv_weight, o_weight):
    from concourse.bass_utils import run_bass_kernel_spmd

    in_maps = _prep_in_maps(x, rotary, qkv_weight, o_weight)
    nc = _get_program()
    res = run_bass_kernel_spmd(nc, in_maps, core_ids=list(range(NCORES)))
    acc = res.results[0]["out"].astype(np.float32)
    for c in range(1, NCORES):
        acc = acc + res.results[c]["out"].astype(np.float32)
    return acc


# revision 58
# speedup vs baseline: 1.0043x; 1.0043x over previous
"""Megatron-style tensor-parallel causal attention (BitLinear qkv/o) on 8 TRN2 cores.

Sharding: each core owns 2 of 16 heads (qkv_weight rows) and the matching
256 o_weight columns. x/rotary replicated; partial outputs summed on host.

All SBUF data is f16 (halves DMA + enables DVE 2x modes); PSUM stays fp32.
Causal mask is folded into the score PSUM via an identity-lhsT matmul adding
-60 to masked entries before exp. Softmax denominator uses the all-ones
lhsT matmul (broadcast rows), normalization on DVE before the out-proj.
Emission order proj(b0), proj(b1), attn(b0), attn(b1) keeps the PE busy:
RoPE(b0) on DVE overlaps proj(b1) matmuls, attention overlaps nothing it
needs.
"""

import math

import numpy as np

EPS = 1e-5
NUM_HEADS = 16
HEAD_DIM = 128
B, S, H = 2, 2048, 2048
NCORES = 8
HPC = NUM_HEADS // NCORES        # heads per core = 2
FPC = 3 * HPC * HEAD_DIM         # qkv features per core = 768
P = 128
NHT = H // P                     # 16 h_in tiles
CH = 512                         # proj token chunk
NCH = S // CH                    # 4 chunks per batch
QC = 256                         # attention q chunk
NQC = S // QC                    # 8
MASKV = -60.0


def _build_program():
    import concourse.bacc as bacc
    import concourse.mybir as mybir
    import concourse.tile as tile

    f32 = mybir.dt.float32
    f16 = mybir.dt.float16
    AF = mybir.ActivationFunctionType

    nc = bacc.Bacc(None, target_bir_lowering=False)

    xt = nc.dram_tensor("xt", [B, H, S], f16, kind="ExternalInput")
    wqkv = nc.dram_tensor("wqkv", [H, FPC], f16, kind="ExternalInput")
    wo = nc.dram_tensor("wo", [HPC * HEAD_DIM, H], f16, kind="ExternalInput")
    cos_t = nc.dram_tensor("cos_t", [P, S], f16, kind="ExternalInput")
    sin_s = nc.dram_tensor("sin_s", [P, S], f16, kind="ExternalInput")
    # aux: [0:512) mask pair (B0|B1), [512:640) identity, [640:768) ones
    aux = nc.dram_tensor("aux", [P, 832], f16, kind="ExternalInput")
    out = nc.dram_tensor("out", [B, S, H], f16, kind="ExternalOutput")

    with tile.TileContext(nc) as tc:
        with tc.tile_pool(name="const", bufs=1) as cpool:
            # first proj chunk's x and the first weight slice lead the DMA
            # queue so the PE starts ~9us in instead of ~24us.
            w_sb = cpool.tile([P, NHT, FPC], f16)
            wre = wqkv.rearrange("(t p) f -> p t f", p=P)
            nc.sync.dma_start(w_sb[:, 0:4, :], wre[:, 0:4, :])

            with (
                tc.tile_pool(name="qk", bufs=2) as qkpool,
                tc.tile_pool(name="vv", bufs=2) as vpool,
                tc.tile_pool(name="work", bufs=2) as wpool,
                tc.tile_pool(name="attn", bufs=3) as apool,
                tc.tile_pool(name="outp", bufs=3) as opool,
                tc.psum_pool(name="pproj", bufs=2) as pps,
                tc.psum_pool(name="pop", bufs=2) as opps,
            ):
                # batch-0 leads with two small 256-token chunks so the first
                # proj matmuls start while weights are still streaming in
                chunks = {
                    0: [(0, 256), (256, 256), (512, 512), (1024, 512), (1536, 512)],
                    1: [(c * CH, CH) for c in range(NCH)],
                }
                # startup: interleave w and first-bite x by h-group
                xt0 = wpool.tile([P, NHT, 256], f16, tag="xt")
                xre0 = xt[0, :, 0:256].rearrange("(t p) c -> p t c", p=P)
                nc.sync.dma_start(xt0[:, 0:4, :], xre0[:, 0:4, :])
                for hgrp in range(1, 4):
                    nc.sync.dma_start(
                        w_sb[:, 4 * hgrp : 4 * (hgrp + 1), :],
                        wre[:, 4 * hgrp : 4 * (hgrp + 1), :],
                    )
                    nc.sync.dma_start(
                        xt0[:, 4 * hgrp : 4 * (hgrp + 1), :],
                        xre0[:, 4 * hgrp : 4 * (hgrp + 1), :],
                    )
                # second bite before the (later-needed) constants
                xt1 = wpool.tile([P, NHT, 256], f16, tag="xt")
                nc.sync.dma_start(
                    xt1[:], xt[0, :, 256:512].rearrange("(t p) c -> p t c", p=P)
                )
                wo_sb = cpool.tile([P, HPC, H], f16)
                nc.sync.dma_start(wo_sb[:], wo.rearrange("(t p) o -> p t o", p=P))
                aux_sb = cpool.tile([P, 832], f16)
                nc.sync.dma_start(aux_sb[:], aux[:])
                rot_sb = cpool.tile([P, 2 * S], f16)
                nc.sync.dma_start(rot_sb[:, 0:S], cos_t[:])
                nc.sync.dma_start(rot_sb[:, S : 2 * S], sin_s[:])

                msk = aux_sb[:, 0:512]          # [k,128] x (B0|B1) for diag pair
                iden = aux_sb[:, 512:640]       # identity
                ones = aux_sb[:, 640:768]       # all-ones
                expb = aux_sb[:, 768:769]       # exp bias column (-8)

                qk_raw = {}   # (b, f) -> raw (pre-rope) tiles
                qk_rope = {}  # (b, f) -> roped tiles
                v_sb = {}     # b -> v tiles [tok_part, ktile, hl*128]
                for b in range(B):
                    for f in range(4):
                        qk_raw[b, f] = qkpool.tile(
                            [P, S], f16, tag=f"qkr{f}", name=f"qkr{f}_{b}"
                        )
                        qk_rope[b, f] = qkpool.tile(
                            [P, S], f16, tag=f"qkf{f}", name=f"qkf{f}_{b}"
                        )
                    v_sb[b] = vpool.tile(
                        [P, (S // P) * 2 * P], f16, tag="v", name=f"v_{b}"
                    )

                # ---------------- projection (+rope) -----------------------
                xt_pre = {}

                def prefetch_xt(b, ci):
                    t0c, W = chunks[b][ci]
                    t = wpool.tile(
                        [P, NHT, W], f16, tag="xt", name=f"xt_{b}_{ci}"
                    )
                    nc.sync.dma_start(
                        t[:],
                        xt[b, :, t0c : t0c + W].rearrange("(t p) c -> p t c", p=P),
                    )
                    xt_pre[b, ci] = t

                def proj_chunk(b, ci):
                    t0c, W = chunks[b][ci]
                    if True:
                        if b == 0 and ci == 0:
                            xt_sb = xt0
                        elif b == 0 and ci == 1:
                            xt_sb = xt1
                        elif (b, ci) in xt_pre:
                            xt_sb = xt_pre.pop((b, ci))
                        else:
                            xt_sb = wpool.tile(
                                [P, NHT, W], f16,
                                tag="xt", name=f"xt_{b}_{ci}",
                            )
                            nc.sync.dma_start(
                                xt_sb[:],
                                xt[b, :, t0c : t0c + W].rearrange(
                                    "(t p) c -> p t c", p=P
                                ),
                            )
                        # q0,q1,k0,k1 : [feat, tok]
                        for f in range(4):
                            ps = pps.tile([P, W], f32, tag="proj", name=f"ps{b}_{ci}_{f}")
                            for h in range(NHT):
                                nc.tensor.matmul(
                                    ps[:],
                                    lhsT=w_sb[:, h, f * P : (f + 1) * P],
                                    rhs=xt_sb[:, h, :],
                                    start=(h == 0),
                                    stop=(h == NHT - 1),
                                )
                            if f % 2 == 0:
                                nc.scalar.copy(
                                    qk_raw[b, f][:, t0c : t0c + W], ps[:]
                                )
                            else:
                                nc.vector.tensor_copy(
                                    qk_raw[b, f][:, t0c : t0c + W], ps[:]
                                )
                        # v: [tok, feat] two tok-subs per psum tile
                        for half in range(W // 256):
                            psv = pps.tile(
                                [P, 512], f32, tag="proj", name=f"psv{b}_{ci}_{half}"
                            )
                            for sub in range(2):
                                tsub = half * 2 + sub
                                for h in range(NHT):
                                    nc.tensor.matmul(
                                        psv[:, sub * 2 * P : (sub + 1) * 2 * P],
                                        lhsT=xt_sb[:, h, tsub * P : (tsub + 1) * P],
                                        rhs=w_sb[:, h, 4 * P : 6 * P],
                                        start=(h == 0),
                                        stop=(h == NHT - 1),
                                    )
                            kt0 = t0c // P + half * 2
                            if half == 0:
                                nc.scalar.copy(
                                    v_sb[b][:, kt0 * 2 * P : (kt0 + 2) * 2 * P], psv[:]
                                )
                            else:
                                nc.vector.tensor_copy(
                                    v_sb[b][:, kt0 * 2 * P : (kt0 + 2) * 2 * P], psv[:]
                                )
                def rope_piece(b, pi):
                    # rope one proj chunk's span; runs on DVE under the next
                    # chunk's proj matmuls
                    t0c, W = chunks[b][pi]
                    for f in range(4):
                        raw = qk_raw[b, f]
                        qsw = wpool.tile(
                            [P, W], f16, tag="qsw", name=f"qsw{b}_{pi}_{f}"
                        )
                        nc.sync.dma_start(
                            qsw[0:64, :], raw[64:128, t0c : t0c + W]
                        )
                        nc.sync.dma_start(
                            qsw[64:128, :], raw[0:64, t0c : t0c + W]
                        )
                        m1 = wpool.tile(
                            [P, W], f16, tag="m1", name=f"m1{b}_{pi}_{f}"
                        )
                        nc.vector.tensor_mul(
                            m1[:], raw[:, t0c : t0c + W], rot_sb[:, t0c : t0c + W]
                        )
                        nc.vector.tensor_mul(
                            qsw[:], qsw[:], rot_sb[:, S + t0c : S + t0c + W]
                        )
                        nc.vector.tensor_add(
                            qk_rope[b, f][:, t0c : t0c + W], m1[:], qsw[:]
                        )

                # ---------------- attention + out-proj ----------------------
                # The last k-tile of each q-chunk only covers q[128:256)
                # (ragged trim). Denominator: full pairs are pre-summed on DVE
                # (halves the ones-matmul rows); the ones-matmul for pair g is
                # deferred until after pair g+1's attn*v so the PE never waits
                # on the DVE add.
                def attn_unit(b, qc, hl):
                    q_t = qk_rope[b, hl]
                    k_t = qk_rope[b, 2 + hl]
                    qs = q_t[:, qc * QC : (qc + 1) * QC]
                    qs_hi = q_t[:, qc * QC + P : (qc + 1) * QC]
                    yt = opps.tile([P, 512], f32, tag="op", name=f"yt{b}_{qc}_{hl}", bufs=5)
                    sm = opps.tile([P, QC], f32, tag="sum", name=f"sm{b}_{qc}_{hl}", bufs=1)
                    pend = None       # deferred exs tile for the ones-matmul
                    sum_started = False

                    def ones_mm(rhs_ap, region, stop):
                        nonlocal sum_started
                        nc.tensor.matmul(
                            sm[:, region[0] : region[1]],
                            lhsT=ones,
                            rhs=rhs_ap,
                            start=not sum_started,
                            stop=stop,
                        )
                        sum_started = True

                    def emit_scores(g):
                        diag = g == qc
                        sc = opps.tile(
                            [P, 2 * QC], f32, tag="op", bufs=5,
                            name=f"sc{b}_{qc}_{hl}_{g}",
                        )
                        nc.tensor.matmul(
                            sc[:, 0:QC],
                            lhsT=k_t[:, 2 * g * P : (2 * g + 1) * P],
                            rhs=qs,
                            start=True,
                            stop=not diag,
                        )
                        if diag:
                            # only the left [128,128] of this tile is masked
                            nc.tensor.matmul(
                                sc[:, 0:P], lhsT=iden, rhs=msk[:, 0:P],
                                start=False, stop=True,
                            )
                            nc.tensor.matmul(
                                sc[:, QC : QC + P],
                                lhsT=k_t[:, (2 * g + 1) * P : (2 * g + 2) * P],
                                rhs=qs_hi,
                                start=True,
                                stop=False,
                            )
                            nc.tensor.matmul(
                                sc[:, QC : QC + P], lhsT=iden, rhs=msk[:, 0:P],
                                start=False, stop=True,
                            )
                        else:
                            nc.tensor.matmul(
                                sc[:, QC : 2 * QC],
                                lhsT=k_t[:, (2 * g + 1) * P : (2 * g + 2) * P],
                                rhs=qs,
                                start=True,
                                stop=True,
                            )
                        return sc

                    # 3-stage pipeline: scores(g+2) and exp(g+1) run ahead of
                    # attn*v(g), so the PE never waits on the Activation
                    # engine's exp. Denominator adds (DVE) get a full
                    # iteration of slack before their ones-matmul.
                    exd = {}   # g -> (ex tile, exs tile or None)

                    def emit_exp(g):
                        diag = g == qc
                        scw = 2 * QC if not diag else QC + P
                        ex = apool.tile([P, scw], f16, tag="ex")
                        nc.scalar.activation(
                            ex[:], scd[g][:, 0:scw], AF.Exp, bias=expb
                        )
                        exs = None
                        if not diag:
                            exs = apool.tile([P, QC], f16, tag="exs")
                            nc.vector.tensor_add(
                                exs[:], ex[:, 0:QC], ex[:, QC : 2 * QC]
                            )
                        exd[g] = (ex, exs)

                    def emit_av(g):
                        diag = g == qc
                        scw = 2 * QC if not diag else QC + P
                        ex, exs = exd.pop(g)
                        v0 = 2 * g * 2 * P + hl * P
                        nc.tensor.matmul(
                            yt[:, 0:QC],
                            lhsT=v_sb[b][:, v0 : v0 + P],
                            rhs=ex[:, 0:QC],
                            start=(g == 0),
                            stop=False,
                        )
                        v1 = (2 * g + 1) * 2 * P + hl * P
                        nc.tensor.matmul(
                            yt[:, P:QC] if diag else yt[:, 0:QC],
                            lhsT=v_sb[b][:, v1 : v1 + P],
                            rhs=ex[:, QC:scw],
                            start=False,
                            stop=diag,
                        )
                        if not diag:
                            ones_mm(exs[:], (0, QC), stop=False)
                        else:
                            ones_mm(ex[:, 0:QC], (0, QC), stop=False)
                            ones_mm(ex[:, QC : QC + P], (P, QC), stop=True)

                    scd = {0: emit_scores(0)}
                    if qc >= 1:
                        scd[1] = emit_scores(1)
                    for g in range(qc + 1):
                        emit_exp(g)
                        if g + 2 <= qc:
                            scd[g + 2] = emit_scores(g + 2)
                        if g >= 1:
                            emit_av(g - 1)
                    emit_av(qc)
                    recip = apool.tile([P, QC], f32, tag="rc")
                    nc.vector.reciprocal(recip[:], sm[:])
                    y = apool.tile([P, QC], f16, tag=f"yn{hl}")
                    nc.vector.tensor_mul(y[:], yt[:, 0:QC], recip[:])
                    return y

                def oproj_part(b, qc, yn, quarters, os_sb):
                    # sub-interleaved so each quarter's PSUM drain overlaps
                    # the other sub's matmuls
                    for quarter in quarters:
                        for sub in range(2):
                            ops = opps.tile([P, 512], f32, tag="op", bufs=5)
                            for hl in range(2):
                                nc.tensor.matmul(
                                    ops[:],
                                    lhsT=yn[hl][:, sub * P : (sub + 1) * P],
                                    rhs=wo_sb[:, hl, quarter * 512 : (quarter + 1) * 512],
                                    start=(hl == 0),
                                    stop=(hl == 1),
                                )
                            if (quarter * 2 + sub) % 2 == 0:
                                nc.scalar.copy(
                                    os_sb[sub][:, quarter * 512 : (quarter + 1) * 512],
                                    ops[:],
                                )
                            else:
                                nc.vector.tensor_copy(
                                    os_sb[sub][:, quarter * 512 : (quarter + 1) * 512],
                                    ops[:],
                                )

                def oproj_alloc(b, qc):
                    return [
                        opool.tile([P, H], f16, tag="os", name=f"os{b}_{qc}_{s}")
                        for s in range(2)
                    ]

                def oproj_flush(b, qc, os_sb):
                    for sub in range(2):
                        t0 = qc * QC + sub * P
                        nc.sync.dma_start(out[b, t0 : t0 + P, :], os_sb[sub][:])

                pending = None

                def attn_step(b, qc):
                    # previous chunk's out-proj lands in two half-bursts
                    # around this chunk's second head-unit: PE work that
                    # hides the normalize chain and spreads evacuations
                    nonlocal pending
                    y0 = attn_unit(b, qc, 0)
                    if pending is not None:
                        pb, pqc, pyn = pending
                        osb = oproj_alloc(pb, pqc)
                        oproj_part(pb, pqc, pyn, (0, 1), osb)
                    y1 = attn_unit(b, qc, 1)
                    if pending is not None:
                        oproj_part(pb, pqc, pyn, (2, 3), osb)
                        oproj_flush(pb, pqc, osb)
                    pending = (b, qc, [y0, y1])

                # fully pipelined schedule: every attention unit runs in a
                # projection window as soon as causally possible, so the
                # exp/evac load always has proj matmuls to hide under and
                # there is no attention-only tail
                b0_units = {0: [], 1: [(0, 0)], 2: [(0, 1), (0, 2), (0, 3)],
                            3: [(0, 4), (0, 5)], 4: [(0, 6)]}
                b1_units = {0: [(0, 7), (1, 0), (1, 1)], 1: [(1, 2), (1, 3)],
                            2: [(1, 4), (1, 5)], 3: [(1, 6), (1, 7)]}
                for pi in range(len(chunks[0])):
                    proj_chunk(0, pi)
                    if pi + 1 < len(chunks[0]):
                        if pi + 1 >= 2:
                            prefetch_xt(0, pi + 1)
                    else:
                        prefetch_xt(1, 0)
                    rope_piece(0, pi)
                    for ub, uqc in b0_units[pi]:
                        attn_step(ub, uqc)
                for ci in range(len(chunks[1])):
                    proj_chunk(1, ci)
                    if ci + 1 < len(chunks[1]):
                        prefetch_xt(1, ci + 1)
                    rope_piece(1, ci)
                    for ub, uqc in b1_units[ci]:
                        attn_step(ub, uqc)
                pb, pqc, pyn = pending
                osb = oproj_alloc(pb, pqc)
                oproj_part(pb, pqc, pyn, (0, 1), osb)
                oproj_part(pb, pqc, pyn, (2, 3), osb)
                oproj_flush(pb, pqc, osb)
    nc.finalize()
    return nc


_NC_CACHE = None


def _get_program():
    global _NC_CACHE
    if _NC_CACHE is None:
        _NC_CACHE = _build_program()
    return _NC_CACHE


def _prep_in_maps(x, rotary, qkv_weight, o_weight):
    import jax
    import ml_dtypes

    bf = np.float16
    cpu = jax.devices("cpu")[0]
    with jax.default_device(cpu):
        import jax.numpy as jnp

        sq = jnp.mean(jnp.abs(jnp.asarray(qkv_weight)))
        wq_q = np.asarray(jnp.round(jnp.asarray(qkv_weight) / (sq + EPS)), np.float32)
        so = jnp.mean(jnp.abs(jnp.asarray(o_weight)))
        wo_q = np.asarray(jnp.round(jnp.asarray(o_weight) / (so + EPS)), np.float32)
        sq = float(sq)
        so = float(so)

    xt = np.ascontiguousarray(x.transpose(0, 2, 1)).astype(bf)
    cos_t = np.ascontiguousarray(rotary[1].T).astype(bf)
    sin_t = np.ascontiguousarray(rotary[0].T).astype(np.float32)
    sin_s = sin_t.copy()
    sin_s[:64] *= -1.0
    sin_s = sin_s.astype(bf)

    # aux: mask pair for the diagonal k-tile pair, identity, ones
    kk = np.arange(P)[:, None]
    qq = np.arange(QC)[None, :]
    aux = np.zeros((P, 832), np.float32)
    aux[:, 0:QC] = np.where(qq < kk, MASKV, 0.0)          # B0: k-tile 2qc
    aux[:, QC : 2 * QC] = np.where(qq < kk + P, MASKV, 0.0)  # B1: k-tile 2qc+1
    aux[:, 512:640] = np.eye(P)
    aux[:, 640:768] = 1.0
    aux[:, 768] = -8.0
    aux = aux.astype(bf)

    # fp16 scaling: sqrt(sm_scale) on BOTH q and k weights (scores land fully
    # scaled in PSUM, masks are in post-scale units); final o-scale folded
    # into v weights (keeps every fp16 tensor in normal range; o_weight stays
    # exactly ternary in fp16).
    alpha = np.float32(math.sqrt(sq * sq / math.sqrt(HEAD_DIM)))
    final_scale = np.float32(sq * so)

    in_maps = []
    for c in range(NCORES):
        rows = []
        for part in range(3):  # q, k, v blocks of qkv_weight
            for hl in range(HPC):
                g = HPC * c + hl
                blk = wq_q[part * H + g * HEAD_DIM : part * H + (g + 1) * HEAD_DIM]
                if part < 2:
                    blk = blk * alpha
                else:
                    blk = blk * final_scale
                rows.append(blk)
        wqkv_c = np.ascontiguousarray(np.concatenate(rows, axis=0).T).astype(bf)
        wo_c = np.ascontiguousarray(
            wo_q[:, c * FPC // 3 : (c + 1) * FPC // 3].T
        ).astype(bf)
        in_maps.append(
            {
                "xt": xt,
                "wqkv": wqkv_c,
                "wo": wo_c,
                "cos_t": cos_t,
                "sin_s": sin_s,
                "aux": aux,
            }
        )
    return in_maps


def kernel(x, rotary, qkv_weight, o_weight):
    from concourse.bass_utils import run_bass_kernel_spmd

    in_maps = _prep_in_maps(x, rotary, qkv_weight, o_weight)
    nc = _get_program()
    res = run_bass_kernel_spmd(nc, in_maps, core_ids=list(range(NCORES)))
    acc = res.results[0]["out"].astype(np.float32)
    for c in range(1, NCORES):
        acc = acc + res.results[c]["out"].astype(np.float32)
    return acc


# revision 60
# speedup vs baseline: 1.0087x; 1.0043x over previous
"""Megatron-style tensor-parallel causal attention (BitLinear qkv/o) on 8 TRN2 cores.

Sharding: each core owns 2 of 16 heads (qkv_weight rows) and the matching
256 o_weight columns. x/rotary replicated; partial outputs summed on host.

All SBUF data is f16 (halves DMA + enables DVE 2x modes); PSUM stays fp32.
Causal mask is folded into the score PSUM via an identity-lhsT matmul adding
-60 to masked entries before exp. Softmax denominator uses the all-ones
lhsT matmul (broadcast rows), normalization on DVE before the out-proj.
Emission order proj(b0), proj(b1), attn(b0), attn(b1) keeps the PE busy:
RoPE(b0) on DVE overlaps proj(b1) matmuls, attention overlaps nothing it
needs.
"""

import math

import numpy as np

EPS = 1e-5
NUM_HEADS = 16
HEAD_DIM = 128
B, S, H = 2, 2048, 2048
NCORES = 8
HPC = NUM_HEADS // NCORES        # heads per core = 2
FPC = 3 * HPC * HEAD_DIM         # qkv features per core = 768
P = 128
NHT = H // P                     # 16 h_in tiles
CH = 512                         # proj token chunk
NCH = S // CH                    # 4 chunks per batch
QC = 256                         # attention q chunk
NQC = S // QC                    # 8
MASKV = -60.0


def _build_program():
    import concourse.bacc as bacc
    import concourse.mybir as mybir
    import concourse.tile as tile

    f32 = mybir.dt.float32
    f16 = mybir.dt.float16
    AF = mybir.ActivationFunctionType

    nc = bacc.Bacc(None, target_bir_lowering=False)

    xt = nc.dram_tensor("xt", [B, H, S], f16, kind="ExternalInput")
    wqkv = nc.dram_tensor("wqkv", [H, FPC], f16, kind="ExternalInput")
    wo = nc.dram_tensor("wo", [HPC * HEAD_DIM, H], f16, kind="ExternalInput")
    cos_t = nc.dram_tensor("cos_t", [P, S], f16, kind="ExternalInput")
    sin_s = nc.dram_tensor("sin_s", [P, S], f16, kind="ExternalInput")
    # aux: [0:512) mask pair (B0|B1), [512:640) identity, [640:768) ones
    aux = nc.dram_tensor("aux", [P, 832], f16, kind="ExternalInput")
    out = nc.dram_tensor("out", [B, S, H], f16, kind="ExternalOutput")

    with tile.TileContext(nc) as tc:
        with tc.tile_pool(name="const", bufs=1) as cpool:
            # first proj chunk's x and the first weight slice lead the DMA
            # queue so the PE starts ~9us in instead of ~24us.
            w_sb = cpool.tile([P, NHT, FPC], f16)
            wre = wqkv.rearrange("(t p) f -> p t f", p=P)
            nc.sync.dma_start(w_sb[:, 0:4, :], wre[:, 0:4, :])

            with (
                tc.tile_pool(name="qk", bufs=2) as qkpool,
                tc.tile_pool(name="vv", bufs=2) as vpool,
                tc.tile_pool(name="work", bufs=2) as wpool,
                tc.tile_pool(name="attn", bufs=3) as apool,
                tc.tile_pool(name="outp", bufs=3) as opool,
                tc.psum_pool(name="pproj", bufs=2) as pps,
                tc.psum_pool(name="pop", bufs=2) as opps,
            ):
                # batch-0 leads with two small 256-token chunks so the first
                # proj matmuls start while weights are still streaming in
                chunks = {
                    0: [(0, 256), (256, 256), (512, 512), (1024, 512), (1536, 512)],
                    1: [(c * CH, CH) for c in range(NCH)],
                }
                # startup: interleave w and first-bite x by h-group
                xt0 = wpool.tile([P, NHT, 256], f16, tag="xt")
                xre0 = xt[0, :, 0:256].rearrange("(t p) c -> p t c", p=P)
                nc.sync.dma_start(xt0[:, 0:4, :], xre0[:, 0:4, :])
                for hgrp in range(1, 4):
                    nc.sync.dma_start(
                        w_sb[:, 4 * hgrp : 4 * (hgrp + 1), :],
                        wre[:, 4 * hgrp : 4 * (hgrp + 1), :],
                    )
                    nc.sync.dma_start(
                        xt0[:, 4 * hgrp : 4 * (hgrp + 1), :],
                        xre0[:, 4 * hgrp : 4 * (hgrp + 1), :],
                    )
                # second bite before the (later-needed) constants
                xt1 = wpool.tile([P, NHT, 256], f16, tag="xt")
                nc.sync.dma_start(
                    xt1[:], xt[0, :, 256:512].rearrange("(t p) c -> p t c", p=P)
                )
                wo_sb = cpool.tile([P, HPC, H], f16)
                nc.sync.dma_start(wo_sb[:], wo.rearrange("(t p) o -> p t o", p=P))
                aux_sb = cpool.tile([P, 832], f16)
                nc.sync.dma_start(aux_sb[:], aux[:])
                rot_sb = cpool.tile([P, 2 * S], f16)
                nc.sync.dma_start(rot_sb[:, 0:S], cos_t[:])
                nc.sync.dma_start(rot_sb[:, S : 2 * S], sin_s[:])

                msk = aux_sb[:, 0:512]          # [k,128] x (B0|B1) for diag pair
                iden = aux_sb[:, 512:640]       # identity
                ones = aux_sb[:, 640:768]       # all-ones
                expb = aux_sb[:, 768:769]       # exp bias column (-8)

                qk_raw = {}   # (b, f) -> raw (pre-rope) tiles
                qk_rope = {}  # (b, f) -> roped tiles
                v_sb = {}     # b -> v tiles [tok_part, ktile, hl*128]
                for b in range(B):
                    for f in range(4):
                        qk_raw[b, f] = qkpool.tile(
                            [P, S], f16, tag=f"qkr{f}", name=f"qkr{f}_{b}"
                        )
                        qk_rope[b, f] = qkpool.tile(
                            [P, S], f16, tag=f"qkf{f}", name=f"qkf{f}_{b}"
                        )
                    v_sb[b] = vpool.tile(
                        [P, (S // P) * 2 * P], f16, tag="v", name=f"v_{b}"
                    )

                # ---------------- projection (+rope) -----------------------
                xt_pre = {}

                def prefetch_xt(b, ci):
                    t0c, W = chunks[b][ci]
                    t = wpool.tile(
                        [P, NHT, W], f16, tag="xt", name=f"xt_{b}_{ci}"
                    )
                    nc.sync.dma_start(
                        t[:],
                        xt[b, :, t0c : t0c + W].rearrange("(t p) c -> p t c", p=P),
                    )
                    xt_pre[b, ci] = t

                def proj_chunk(b, ci):
                    t0c, W = chunks[b][ci]
                    if True:
                        if b == 0 and ci == 0:
                            xt_sb = xt0
                        elif b == 0 and ci == 1:
                            xt_sb = xt1
                        elif (b, ci) in xt_pre:
                            xt_sb = xt_pre.pop((b, ci))
                        else:
                            xt_sb = wpool.tile(
                                [P, NHT, W], f16,
                                tag="xt", name=f"xt_{b}_{ci}",
                            )
                            nc.sync.dma_start(
                                xt_sb[:],
                                xt[b, :, t0c : t0c + W].rearrange(
                                    "(t p) c -> p t c", p=P
                                ),
                            )
                        # q0,q1,k0,k1 : [feat, tok]
                        for f in range(4):
                            ps = pps.tile([P, W], f32, tag="proj", name=f"ps{b}_{ci}_{f}")
                            for h in range(NHT):
                                nc.tensor.matmul(
                                    ps[:],
                                    lhsT=w_sb[:, h, f * P : (f + 1) * P],
                                    rhs=xt_sb[:, h, :],
                                    start=(h == 0),
                                    stop=(h == NHT - 1),
                                )
                            if f % 2 == 0:
                                nc.scalar.copy(
                                    qk_raw[b, f][:, t0c : t0c + W], ps[:]
                                )
                            else:
                                nc.vector.tensor_copy(
                                    qk_raw[b, f][:, t0c : t0c + W], ps[:]
                                )
                        # v: [tok, feat] two tok-subs per psum tile
                        for half in range(W // 256):
                            psv = pps.tile(
                                [P, 512], f32, tag="proj", name=f"psv{b}_{ci}_{half}"
                            )
                            for sub in range(2):
                                tsub = half * 2 + sub
                                for h in range(NHT):
                                    nc.tensor.matmul(
                                        psv[:, sub * 2 * P : (sub + 1) * 2 * P],
                                        lhsT=xt_sb[:, h, tsub * P : (tsub + 1) * P],
                                        rhs=w_sb[:, h, 4 * P : 6 * P],
                                        start=(h == 0),
                                        stop=(h == NHT - 1),
                                    )
                            kt0 = t0c // P + half * 2
                            if half == 0:
                                nc.scalar.copy(
                                    v_sb[b][:, kt0 * 2 * P : (kt0 + 2) * 2 * P], psv[:]
                                )
                            else:
                                nc.vector.tensor_copy(
                                    v_sb[b][:, kt0 * 2 * P : (kt0 + 2) * 2 * P], psv[:]
                                )
                def rope_piece(b, pi):
                    # rope one proj chunk's span; runs on DVE under the next
                    # chunk's proj matmuls
                    t0c, W = chunks[b][pi]
                    for f in range(4):
                        raw = qk_raw[b, f]
                        qsw = wpool.tile(
                            [P, W], f16, tag="qsw", name=f"qsw{b}_{pi}_{f}"
                        )
                        nc.sync.dma_start(
                            qsw[0:64, :], raw[64:128, t0c : t0c + W]
                        )
                        nc.sync.dma_start(
                            qsw[64:128, :], raw[0:64, t0c : t0c + W]
                        )
                        m1 = wpool.tile(
                            [P, W], f16, tag="m1", name=f"m1{b}_{pi}_{f}"
                        )
                        nc.vector.tensor_mul(
                            m1[:], raw[:, t0c : t0c + W], rot_sb[:, t0c : t0c + W]
                        )
                        nc.vector.tensor_mul(
                            qsw[:], qsw[:], rot_sb[:, S + t0c : S + t0c + W]
                        )
                        nc.vector.tensor_add(
                            qk_rope[b, f][:, t0c : t0c + W], m1[:], qsw[:]
                        )

                # ---------------- attention + out-proj ----------------------
                # The last k-tile of each q-chunk only covers q[128:256)
                # (ragged trim). Denominator: full pairs are pre-summed on DVE
                # (halves the ones-matmul rows); the ones-matmul for pair g is
                # deferred until after pair g+1's attn*v so the PE never waits
                # on the DVE add.
                def attn_unit(b, qc, hl):
                    q_t = qk_rope[b, hl]
                    k_t = qk_rope[b, 2 + hl]
                    qs = q_t[:, qc * QC : (qc + 1) * QC]
                    qs_hi = q_t[:, qc * QC + P : (qc + 1) * QC]
                    yt = opps.tile([P, 512], f32, tag="op", name=f"yt{b}_{qc}_{hl}", bufs=5)
                    sm = opps.tile([P, QC], f32, tag="sum", name=f"sm{b}_{qc}_{hl}", bufs=1)
                    pend = None       # deferred exs tile for the ones-matmul
                    sum_started = False

                    def ones_mm(rhs_ap, region, stop):
                        nonlocal sum_started
                        nc.tensor.matmul(
                            sm[:, region[0] : region[1]],
                            lhsT=ones,
                            rhs=rhs_ap,
                            start=not sum_started,
                            stop=stop,
                        )
                        sum_started = True

                    def emit_scores(g):
                        diag = g == qc
                        sc = opps.tile(
                            [P, 2 * QC], f32, tag="op", bufs=5,
                            name=f"sc{b}_{qc}_{hl}_{g}",
                        )
                        nc.tensor.matmul(
                            sc[:, 0:QC],
                            lhsT=k_t[:, 2 * g * P : (2 * g + 1) * P],
                            rhs=qs,
                            start=True,
                            stop=not diag,
                        )
                        if diag:
                            # only the left [128,128] of this tile is masked
                            nc.tensor.matmul(
                                sc[:, 0:P], lhsT=iden, rhs=msk[:, 0:P],
                                start=False, stop=True,
                            )
                            nc.tensor.matmul(
                                sc[:, QC : QC + P],
                                lhsT=k_t[:, (2 * g + 1) * P : (2 * g + 2) * P],
                                rhs=qs_hi,
                                start=True,
                                stop=False,
                            )
                            nc.tensor.matmul(
                                sc[:, QC : QC + P], lhsT=iden, rhs=msk[:, 0:P],
                                start=False, stop=True,
                            )
                        else:
                            nc.tensor.matmul(
                                sc[:, QC : 2 * QC],
                                lhsT=k_t[:, (2 * g + 1) * P : (2 * g + 2) * P],
                                rhs=qs,
                                start=True,
                                stop=True,
                            )
                        return sc

                    # 3-stage pipeline: scores(g+2) and exp(g+1) run ahead of
                    # attn*v(g), so the PE never waits on the Activation
                    # engine's exp. Denominator adds (DVE) get a full
                    # iteration of slack before their ones-matmul.
                    exd = {}   # g -> (ex tile, exs tile or None)

                    def emit_exp(g):
                        diag = g == qc
                        scw = 2 * QC if not diag else QC + P
                        ex = apool.tile([P, scw], f16, tag="ex")
                        nc.scalar.activation(
                            ex[:], scd[g][:, 0:scw], AF.Exp, bias=expb
                        )
                        if not diag:
                            exs = apool.tile([P, QC], f16, tag="exs")
                            nc.vector.tensor_add(
                                exs[:], ex[:, 0:QC], ex[:, QC : 2 * QC]
                            )
                        else:
                            # combine the two k-tiles' shared q-half so the
                            # diagonal denominator is two 128-row matmuls
                            exs = apool.tile([P, P], f16, tag="exs")
                            nc.vector.tensor_add(
                                exs[:], ex[:, P:QC], ex[:, QC : QC + P]
                            )
                        exd[g] = (ex, exs)

                    def emit_av(g):
                        diag = g == qc
                        scw = 2 * QC if not diag else QC + P
                        ex, exs = exd.pop(g)
                        v0 = 2 * g * 2 * P + hl * P
                        nc.tensor.matmul(
                            yt[:, 0:QC],
                            lhsT=v_sb[b][:, v0 : v0 + P],
                            rhs=ex[:, 0:QC],
                            start=(g == 0),
                            stop=False,
                        )
                        v1 = (2 * g + 1) * 2 * P + hl * P
                        nc.tensor.matmul(
                            yt[:, P:QC] if diag else yt[:, 0:QC],
                            lhsT=v_sb[b][:, v1 : v1 + P],
                            rhs=ex[:, QC:scw],
                            start=False,
                            stop=diag,
                        )
                        if not diag:
                            ones_mm(exs[:], (0, QC), stop=False)
                        elif qc == 0:
                            # no prior pair zeroed the region: cover all of it
                            ones_mm(ex[:, 0:QC], (0, QC), stop=False)
                            ones_mm(ex[:, QC : QC + P], (P, QC), stop=True)
                        else:
                            ones_mm(ex[:, 0:P], (0, P), stop=False)
                            ones_mm(exs[:], (P, QC), stop=True)

                    scd = {0: emit_scores(0)}
                    if qc >= 1:
                        scd[1] = emit_scores(1)
                    for g in range(qc + 1):
                        emit_exp(g)
                        if g + 2 <= qc:
                            scd[g + 2] = emit_scores(g + 2)
                        if g >= 1:
                            emit_av(g - 1)
                    emit_av(qc)
                    recip = apool.tile([P, QC], f32, tag="rc")
                    nc.vector.reciprocal(recip[:], sm[:])
                    y = apool.tile([P, QC], f16, tag=f"yn{hl}")
                    nc.vector.tensor_mul(y[:], yt[:, 0:QC], recip[:])
                    return y

                def oproj_part(b, qc, yn, quarters, os_sb):
                    # sub-interleaved so each quarter's PSUM drain overlaps
                    # the other sub's matmuls
                    for quarter in quarters:
                        for sub in range(2):
                            ops = opps.tile([P, 512], f32, tag="op", bufs=5)
                            for hl in range(2):
                                nc.tensor.matmul(
                                    ops[:],
                                    lhsT=yn[hl][:, sub * P : (sub + 1) * P],
                                    rhs=wo_sb[:, hl, quarter * 512 : (quarter + 1) * 512],
                                    start=(hl == 0),
                                    stop=(hl == 1),
                                )
                            if (quarter * 2 + sub) % 2 == 0:
                                nc.scalar.copy(
                                    os_sb[sub][:, quarter * 512 : (quarter + 1) * 512],
                                    ops[:],
                                )
                            else:
                                nc.vector.tensor_copy(
                                    os_sb[sub][:, quarter * 512 : (quarter + 1) * 512],
                                    ops[:],
                                )

                def oproj_alloc(b, qc):
                    return [
                        opool.tile([P, H], f16, tag="os", name=f"os{b}_{qc}_{s}")
                        for s in range(2)
                    ]

                def oproj_flush(b, qc, os_sb):
                    for sub in range(2):
                        t0 = qc * QC + sub * P
                        nc.sync.dma_start(out[b, t0 : t0 + P, :], os_sb[sub][:])

                pending = None

                def attn_step(b, qc):
                    # previous chunk's out-proj lands in two half-bursts
                    # around this chunk's second head-unit: PE work that
                    # hides the normalize chain and spreads evacuations
                    nonlocal pending
                    y0 = attn_unit(b, qc, 0)
                    if pending is not None:
                        pb, pqc, pyn = pending
                        osb = oproj_alloc(pb, pqc)
                        oproj_part(pb, pqc, pyn, (0, 1), osb)
                    y1 = attn_unit(b, qc, 1)
                    if pending is not None:
                        oproj_part(pb, pqc, pyn, (2, 3), osb)
                        oproj_flush(pb, pqc, osb)
                    pending = (b, qc, [y0, y1])

                # fully pipelined schedule: every attention unit runs in a
                # projection window as soon as causally possible, so the
                # exp/evac load always has proj matmuls to hide under and
                # there is no attention-only tail
                b0_units = {0: [], 1: [(0, 0)], 2: [(0, 1), (0, 2), (0, 3)],
                            3: [(0, 4), (0, 5)], 4: [(0, 6)]}
                b1_units = {0: [(0, 7), (1, 0), (1, 1)], 1: [(1, 2), (1, 3)],
                            2: [(1, 4), (1, 5)], 3: [(1, 6), (1, 7)]}
                for pi in range(len(chunks[0])):
                    proj_chunk(0, pi)
                    if pi + 1 < len(chunks[0]):
                        if pi + 1 >= 2:
                            prefetch_xt(0, pi + 1)
                    else:
                        prefetch_xt(1, 0)
                    rope_piece(0, pi)
                    for ub, uqc in b0_units[pi]:
                        attn_step(ub, uqc)
                for ci in range(len(chunks[1])):
                    proj_chunk(1, ci)
                    if ci + 1 < len(chunks[1]):
                        prefetch_xt(1, ci + 1)
                    rope_piece(1, ci)
                    for ub, uqc in b1_units[ci]:
                        attn_step(ub, uqc)
                pb, pqc, pyn = pending
                osb = oproj_alloc(pb, pqc)
                oproj_part(pb, pqc, pyn, (0, 1), osb)
                oproj_part(pb, pqc, pyn, (2, 3), osb)
                oproj_flush(pb, pqc, osb)
    nc.finalize()
    return nc


_NC_CACHE = None


def _get_program():
    global _NC_CACHE
    if _NC_CACHE is None:
        _NC_CACHE = _build_program()
    return _NC_CACHE


def _prep_in_maps(x, rotary, qkv_weight, o_weight):
    import jax
    import ml_dtypes

    bf = np.float16
    cpu = jax.devices("cpu")[0]
    with jax.default_device(cpu):
        import jax.numpy as jnp

        sq = jnp.mean(jnp.abs(jnp.asarray(qkv_weight)))
        wq_q = np.asarray(jnp.round(jnp.asarray(qkv_weight) / (sq + EPS)), np.float32)
        so = jnp.mean(jnp.abs(jnp.asarray(o_weight)))
        wo_q = np.asarray(jnp.round(jnp.asarray(o_weight) / (so + EPS)), np.float32)
        sq = float(sq)
        so = float(so)

    xt = np.ascontiguousarray(x.transpose(0, 2, 1)).astype(bf)
    cos_t = np.ascontiguousarray(rotary[1].T).astype(bf)
    sin_t = np.ascontiguousarray(rotary[0].T).astype(np.float32)
    sin_s = sin_t.copy()
    sin_s[:64] *= -1.0
    sin_s = sin_s.astype(bf)

    # aux: mask pair for the diagonal k-tile pair, identity, ones
    kk = np.arange(P)[:, None]
    qq = np.arange(QC)[None, :]
    aux = np.zeros((P, 832), np.float32)
    aux[:, 0:QC] = np.where(qq < kk, MASKV, 0.0)          # B0: k-tile 2qc
    aux[:, QC : 2 * QC] = np.where(qq < kk + P, MASKV, 0.0)  # B1: k-tile 2qc+1
    aux[:, 512:640] = np.eye(P)
    aux[:, 640:768] = 1.0
    aux[:, 768] = -8.0
    aux = aux.astype(bf)

    # fp16 scaling: sqrt(sm_scale) on BOTH q and k weights (scores land fully
    # scaled in PSUM, masks are in post-scale units); final o-scale folded
    # into v weights (keeps every fp16 tensor in normal range; o_weight stays
    # exactly ternary in fp16).
    alpha = np.float32(math.sqrt(sq * sq / math.sqrt(HEAD_DIM)))
    final_scale = np.float32(sq * so)

    in_maps = []
    for c in range(NCORES):
        rows = []
        for part in range(3):  # q, k, v blocks of qkv_weight
            for hl in range(HPC):
                g = HPC * c + hl
                blk = wq_q[part * H + g * HEAD_DIM : part * H + (g + 1) * HEAD_DIM]
                if part < 2:
                    blk = blk * alpha
                else:
                    blk = blk * final_scale
                rows.append(blk)
        wqkv_c = np.ascontiguousarray(np.concatenate(rows, axis=0).T).astype(bf)
        wo_c = np.ascontiguousarray(
            wo_q[:, c * FPC // 3 : (c + 1) * FPC // 3].T
        ).astype(bf)
        in_maps.append(
            {
                "xt": xt,
                "wqkv": wqkv_c,
                "wo": wo_c,
                "cos_t": cos_t,
                "sin_s": sin_s,
                "aux": aux,
            }
        )
    return in_maps


def kernel(x, rotary, qkv_weight, o_weight):
    from concourse.bass_utils import run_bass_kernel_spmd

    in_maps = _prep_in_maps(x, rotary, qkv_weight, o_weight)
    nc = _get_program()
    res = run_bass_kernel_spmd(nc, in_maps, core_ids=list(range(NCORES)))
    acc = res.results[0]["out"].astype(np.float32)
    for c in range(1, NCORES):
        acc = acc + res.results[c]["out"].astype(np.float32)
    return acc


# revision 62
# speedup vs baseline: 1.0218x; 1.0131x over previous
"""Megatron-style tensor-parallel causal attention (BitLinear qkv/o) on 8 TRN2 cores.

Sharding: each core owns 2 of 16 heads (qkv_weight rows) and the matching
256 o_weight columns. x/rotary replicated; partial outputs summed on host.

All SBUF data is f16 (halves DMA + enables DVE 2x modes); PSUM stays fp32.
Causal mask is folded into the score PSUM via an identity-lhsT matmul adding
-60 to masked entries before exp. Softmax denominator uses the all-ones
lhsT matmul (broadcast rows), normalization on DVE before the out-proj.
Emission order proj(b0), proj(b1), attn(b0), attn(b1) keeps the PE busy:
RoPE(b0) on DVE overlaps proj(b1) matmuls, attention overlaps nothing it
needs.
"""

import math

import numpy as np

EPS = 1e-5
NUM_HEADS = 16
HEAD_DIM = 128
B, S, H = 2, 2048, 2048
NCORES = 8
HPC = NUM_HEADS // NCORES        # heads per core = 2
FPC = 3 * HPC * HEAD_DIM         # qkv features per core = 768
P = 128
NHT = H // P                     # 16 h_in tiles
CH = 512                         # proj token chunk
NCH = S // CH                    # 4 chunks per batch
QC = 256                         # attention q chunk
NQC = S // QC                    # 8
MASKV = -60.0


def _build_program():
    import concourse.bacc as bacc
    import concourse.mybir as mybir
    import concourse.tile as tile

    f32 = mybir.dt.float32
    f16 = mybir.dt.float16
    AF = mybir.ActivationFunctionType

    nc = bacc.Bacc(None, target_bir_lowering=False)

    xt = nc.dram_tensor("xt", [B, H, S], f16, kind="ExternalInput")
    wqkv = nc.dram_tensor("wqkv", [H, FPC], f16, kind="ExternalInput")
    wo = nc.dram_tensor("wo", [HPC * HEAD_DIM, H], f16, kind="ExternalInput")
    cos_t = nc.dram_tensor("cos_t", [P, S], f16, kind="ExternalInput")
    sin_s = nc.dram_tensor("sin_s", [P, S], f16, kind="ExternalInput")
    # aux: [0:512) mask pair (B0|B1), [512:640) identity, [640:768) ones
    aux = nc.dram_tensor("aux", [P, 832], f16, kind="ExternalInput")
    out = nc.dram_tensor("out", [B, S, H], f16, kind="ExternalOutput")

    with tile.TileContext(nc) as tc:
        with tc.tile_pool(name="const", bufs=1) as cpool:
            # first proj chunk's x and the first weight slice lead the DMA
            # queue so the PE starts ~9us in instead of ~24us.
            w_sb = cpool.tile([P, NHT, FPC], f16)
            wre = wqkv.rearrange("(t p) f -> p t f", p=P)
            nc.sync.dma_start(w_sb[:, 0:4, :], wre[:, 0:4, :])

            with (
                tc.tile_pool(name="qk", bufs=2) as qkpool,
                tc.tile_pool(name="vv", bufs=2) as vpool,
                tc.tile_pool(name="work", bufs=2) as wpool,
                tc.tile_pool(name="attn", bufs=3) as apool,
                tc.tile_pool(name="outp", bufs=3) as opool,
                tc.psum_pool(name="pproj", bufs=2) as pps,
                tc.psum_pool(name="pop", bufs=2) as opps,
            ):
                # batch-0 leads with two small 256-token chunks so the first
                # proj matmuls start while weights are still streaming in
                chunks = {
                    0: [(0, 256), (256, 256), (512, 512), (1024, 512), (1536, 512)],
                    1: [(c * CH, CH) for c in range(NCH)],
                }
                # startup: interleave w and first-bite x by h-group
                xt0 = wpool.tile([P, NHT, 256], f16, tag="xt")
                xre0 = xt[0, :, 0:256].rearrange("(t p) c -> p t c", p=P)
                nc.sync.dma_start(xt0[:, 0:4, :], xre0[:, 0:4, :])
                for hgrp in range(1, 4):
                    nc.sync.dma_start(
                        w_sb[:, 4 * hgrp : 4 * (hgrp + 1), :],
                        wre[:, 4 * hgrp : 4 * (hgrp + 1), :],
                    )
                    nc.sync.dma_start(
                        xt0[:, 4 * hgrp : 4 * (hgrp + 1), :],
                        xre0[:, 4 * hgrp : 4 * (hgrp + 1), :],
                    )
                # second bite before the (later-needed) constants
                xt1 = wpool.tile([P, NHT, 256], f16, tag="xt")
                nc.sync.dma_start(
                    xt1[:], xt[0, :, 256:512].rearrange("(t p) c -> p t c", p=P)
                )
                wo_sb = cpool.tile([P, HPC, H], f16)
                nc.sync.dma_start(wo_sb[:], wo.rearrange("(t p) o -> p t o", p=P))
                aux_sb = cpool.tile([P, 832], f16)
                nc.sync.dma_start(aux_sb[:], aux[:])
                rot_sb = cpool.tile([P, 2 * S], f16)
                nc.sync.dma_start(rot_sb[:, 0:S], cos_t[:])
                nc.sync.dma_start(rot_sb[:, S : 2 * S], sin_s[:])

                msk = aux_sb[:, 0:512]          # [k,128] x (B0|B1) for diag pair
                iden = aux_sb[:, 512:640]       # identity
                ones = aux_sb[:, 640:768]       # all-ones
                expb = aux_sb[:, 768:769]       # exp bias column (-8)

                qk_raw = {}   # (b, f) -> raw (pre-rope) tiles
                qk_rope = {}  # (b, f) -> roped tiles
                v_sb = {}     # b -> v tiles [tok_part, ktile, hl*128]
                for b in range(B):
                    for f in range(4):
                        qk_raw[b, f] = qkpool.tile(
                            [P, S], f16, tag=f"qkr{f}", name=f"qkr{f}_{b}"
                        )
                        qk_rope[b, f] = qkpool.tile(
                            [P, S], f16, tag=f"qkf{f}", name=f"qkf{f}_{b}"
                        )
                    v_sb[b] = vpool.tile(
                        [P, (S // P) * 2 * P], f16, tag="v", name=f"v_{b}"
                    )

                # ---------------- projection (+rope) -----------------------
                xt_pre = {}

                def prefetch_xt(b, ci):
                    t0c, W = chunks[b][ci]
                    t = wpool.tile(
                        [P, NHT, W], f16, tag="xt", name=f"xt_{b}_{ci}"
                    )
                    nc.sync.dma_start(
                        t[:],
                        xt[b, :, t0c : t0c + W].rearrange("(t p) c -> p t c", p=P),
                    )
                    xt_pre[b, ci] = t

                def proj_chunk(b, ci):
                    t0c, W = chunks[b][ci]
                    if True:
                        if b == 0 and ci == 0:
                            xt_sb = xt0
                        elif b == 0 and ci == 1:
                            xt_sb = xt1
                        elif (b, ci) in xt_pre:
                            xt_sb = xt_pre.pop((b, ci))
                        else:
                            xt_sb = wpool.tile(
                                [P, NHT, W], f16,
                                tag="xt", name=f"xt_{b}_{ci}",
                            )
                            nc.sync.dma_start(
                                xt_sb[:],
                                xt[b, :, t0c : t0c + W].rearrange(
                                    "(t p) c -> p t c", p=P
                                ),
                            )
                        # q0,q1,k0,k1 : [feat, tok]
                        for f in range(4):
                            ps = pps.tile([P, W], f32, tag="proj", name=f"ps{b}_{ci}_{f}")
                            for h in range(NHT):
                                nc.tensor.matmul(
                                    ps[:],
                                    lhsT=w_sb[:, h, f * P : (f + 1) * P],
                                    rhs=xt_sb[:, h, :],
                                    start=(h == 0),
                                    stop=(h == NHT - 1),
                                )
                            if f % 2 == 0:
                                nc.scalar.copy(
                                    qk_raw[b, f][:, t0c : t0c + W], ps[:]
                                )
                            else:
                                nc.vector.tensor_copy(
                                    qk_raw[b, f][:, t0c : t0c + W], ps[:]
                                )
                        # v: [tok, feat] two tok-subs per psum tile
                        for half in range(W // 256):
                            psv = pps.tile(
                                [P, 512], f32, tag="proj", name=f"psv{b}_{ci}_{half}"
                            )
                            for sub in range(2):
                                tsub = half * 2 + sub
                                for h in range(NHT):
                                    nc.tensor.matmul(
                                        psv[:, sub * 2 * P : (sub + 1) * 2 * P],
                                        lhsT=xt_sb[:, h, tsub * P : (tsub + 1) * P],
                                        rhs=w_sb[:, h, 4 * P : 6 * P],
                                        start=(h == 0),
                                        stop=(h == NHT - 1),
                                    )
                            kt0 = t0c // P + half * 2
                            if half == 0:
                                nc.scalar.copy(
                                    v_sb[b][:, kt0 * 2 * P : (kt0 + 2) * 2 * P], psv[:]
                                )
                            else:
                                nc.vector.tensor_copy(
                                    v_sb[b][:, kt0 * 2 * P : (kt0 + 2) * 2 * P], psv[:]
                                )
                def rope_piece(b, pi):
                    # rope one proj chunk's span; runs on DVE under the next
                    # chunk's proj matmuls
                    t0c, W = chunks[b][pi]
                    for f in range(4):
                        raw = qk_raw[b, f]
                        qsw = wpool.tile(
                            [P, W], f16, tag="qsw", name=f"qsw{b}_{pi}_{f}"
                        )
                        nc.sync.dma_start(
                            qsw[0:64, :], raw[64:128, t0c : t0c + W]
                        )
                        nc.sync.dma_start(
                            qsw[64:128, :], raw[0:64, t0c : t0c + W]
                        )
                        m1 = wpool.tile(
                            [P, W], f16, tag="m1", name=f"m1{b}_{pi}_{f}"
                        )
                        nc.vector.tensor_mul(
                            m1[:], raw[:, t0c : t0c + W], rot_sb[:, t0c : t0c + W]
                        )
                        nc.vector.tensor_mul(
                            qsw[:], qsw[:], rot_sb[:, S + t0c : S + t0c + W]
                        )
                        nc.vector.tensor_add(
                            qk_rope[b, f][:, t0c : t0c + W], m1[:], qsw[:]
                        )

                # ---------------- attention + out-proj ----------------------
                # The last k-tile of each q-chunk only covers q[128:256)
                # (ragged trim). Denominator: full pairs are pre-summed on DVE
                # (halves the ones-matmul rows); the ones-matmul for pair g is
                # deferred until after pair g+1's attn*v so the PE never waits
                # on the DVE add.
                def attn_unit(b, qc, hl):
                    q_t = qk_rope[b, hl]
                    k_t = qk_rope[b, 2 + hl]
                    qs = q_t[:, qc * QC : (qc + 1) * QC]
                    qs_hi = q_t[:, qc * QC + P : (qc + 1) * QC]
                    yt = opps.tile([P, 512], f32, tag="op", name=f"yt{b}_{qc}_{hl}", bufs=5)
                    sm = opps.tile([P, QC], f32, tag="sum", name=f"sm{b}_{qc}_{hl}", bufs=1)
                    pend = None       # deferred exs tile for the ones-matmul
                    sum_started = False

                    def ones_mm(rhs_ap, region, stop):
                        nonlocal sum_started
                        nc.tensor.matmul(
                            sm[:, region[0] : region[1]],
                            lhsT=ones,
                            rhs=rhs_ap,
                            start=not sum_started,
                            stop=stop,
                        )
                        sum_started = True

                    def emit_scores(g):
                        diag = g == qc
                        sc = opps.tile(
                            [P, 2 * QC], f32, tag="op", bufs=5,
                            name=f"sc{b}_{qc}_{hl}_{g}",
                        )
                        nc.tensor.matmul(
                            sc[:, 0:QC],
                            lhsT=k_t[:, 2 * g * P : (2 * g + 1) * P],
                            rhs=qs,
                            start=True,
                            stop=not diag,
                        )
                        if diag:
                            # only the left [128,128] of this tile is masked
                            nc.tensor.matmul(
                                sc[:, 0:P], lhsT=iden, rhs=msk[:, 0:P],
                                start=False, stop=True,
                            )
                            nc.tensor.matmul(
                                sc[:, QC : QC + P],
                                lhsT=k_t[:, (2 * g + 1) * P : (2 * g + 2) * P],
                                rhs=qs_hi,
                                start=True,
                                stop=False,
                            )
                            nc.tensor.matmul(
                                sc[:, QC : QC + P], lhsT=iden, rhs=msk[:, 0:P],
                                start=False, stop=True,
                            )
                        else:
                            nc.tensor.matmul(
                                sc[:, QC : 2 * QC],
                                lhsT=k_t[:, (2 * g + 1) * P : (2 * g + 2) * P],
                                rhs=qs,
                                start=True,
                                stop=True,
                            )
                        return sc

                    # 3-stage pipeline: scores(g+2) and exp(g+1) run ahead of
                    # attn*v(g), so the PE never waits on the Activation
                    # engine's exp. Denominator adds (DVE) get a full
                    # iteration of slack before their ones-matmul.
                    exd = {}   # g -> (ex tile, exs tile or None)

                    def emit_exp(g):
                        diag = g == qc
                        scw = 2 * QC if not diag else QC + P
                        ex = apool.tile([P, scw], f16, tag="ex")
                        nc.scalar.activation(
                            ex[:], scd[g][:, 0:scw], AF.Exp, bias=expb
                        )
                        if not diag:
                            exs = apool.tile([P, QC], f16, tag="exs", bufs=4)
                            nc.vector.tensor_add(
                                exs[:], ex[:, 0:QC], ex[:, QC : 2 * QC]
                            )
                        else:
                            # combine the two k-tiles' shared q-half so the
                            # diagonal denominator is two 128-row matmuls
                            exs = apool.tile([P, P], f16, tag="exs", bufs=4)
                            nc.vector.tensor_add(
                                exs[:], ex[:, P:QC], ex[:, QC : QC + P]
                            )
                        exd[g] = (ex, exs)

                    def emit_av(g):
                        nonlocal qpend, opend
                        diag = g == qc
                        scw = 2 * QC if not diag else QC + P
                        ex, exs = exd.pop(g)
                        v0 = 2 * g * 2 * P + hl * P
                        nc.tensor.matmul(
                            yt[:, 0:QC],
                            lhsT=v_sb[b][:, v0 : v0 + P],
                            rhs=ex[:, 0:QC],
                            start=(g == 0),
                            stop=False,
                        )
                        v1 = (2 * g + 1) * 2 * P + hl * P
                        nc.tensor.matmul(
                            yt[:, P:QC] if diag else yt[:, 0:QC],
                            lhsT=v_sb[b][:, v1 : v1 + P],
                            rhs=ex[:, QC:scw],
                            start=False,
                            stop=diag,
                        )
                        # quad-summed denominator: ones-matmuls run on
                        # pair-of-pair sums, each deferred one iteration so
                        # the PE never waits on the DVE adds
                        if opend is not None:
                            ones_mm(opend[:], (0, QC), stop=False)
                            opend = None
                        if not diag:
                            if qpend is None:
                                qpend = exs
                            else:
                                exq = apool.tile(
                                    [P, QC], f16, tag="exq",
                                    name=f"exq{b}_{qc}_{hl}_{g}",
                                )
                                nc.vector.tensor_add(exq[:], qpend[:], exs[:])
                                qpend = None
                                opend = exq
                        elif qc == 0:
                            # no prior pair zeroed the region: cover all of it
                            ones_mm(ex[:, 0:QC], (0, QC), stop=False)
                            ones_mm(ex[:, QC : QC + P], (P, QC), stop=True)
                        else:
                            if qpend is not None:
                                ones_mm(qpend[:], (0, QC), stop=False)
                                qpend = None
                            ones_mm(ex[:, 0:P], (0, P), stop=False)
                            ones_mm(exs[:], (P, QC), stop=True)

                    qpend = None   # exs awaiting its quad partner
                    opend = None   # quad sum awaiting its ones-matmul
                    scd = {0: emit_scores(0)}
                    if qc >= 1:
                        scd[1] = emit_scores(1)
                    for g in range(qc + 1):
                        emit_exp(g)
                        if g + 2 <= qc:
                            scd[g + 2] = emit_scores(g + 2)
                        if g >= 1:
                            emit_av(g - 1)
                    emit_av(qc)
                    recip = apool.tile([P, QC], f32, tag="rc")
                    nc.vector.reciprocal(recip[:], sm[:])
                    y = apool.tile([P, QC], f16, tag=f"yn{hl}")
                    nc.vector.tensor_mul(y[:], yt[:, 0:QC], recip[:])
                    return y

                def oproj_part(b, qc, yn, quarters, os_sb):
                    # sub-interleaved so each quarter's PSUM drain overlaps
                    # the other sub's matmuls
                    for quarter in quarters:
                        for sub in range(2):
                            ops = opps.tile([P, 512], f32, tag="op", bufs=5)
                            for hl in range(2):
                                nc.tensor.matmul(
                                    ops[:],
                                    lhsT=yn[hl][:, sub * P : (sub + 1) * P],
                                    rhs=wo_sb[:, hl, quarter * 512 : (quarter + 1) * 512],
                                    start=(hl == 0),
                                    stop=(hl == 1),
                                )
                            if (quarter * 2 + sub) % 2 == 0:
                                nc.scalar.copy(
                                    os_sb[sub][:, quarter * 512 : (quarter + 1) * 512],
                                    ops[:],
                                )
                            else:
                                nc.vector.tensor_copy(
                                    os_sb[sub][:, quarter * 512 : (quarter + 1) * 512],
                                    ops[:],
                                )

                def oproj_alloc(b, qc):
                    return [
                        opool.tile([P, H], f16, tag="os", name=f"os{b}_{qc}_{s}")
                        for s in range(2)
                    ]

                def oproj_flush(b, qc, os_sb):
                    for sub in range(2):
                        t0 = qc * QC + sub * P
                        nc.sync.dma_start(out[b, t0 : t0 + P, :], os_sb[sub][:])

                pending = None

                def attn_step(b, qc):
                    # previous chunk's out-proj lands in two half-bursts
                    # around this chunk's second head-unit: PE work that
                    # hides the normalize chain and spreads evacuations
                    nonlocal pending
                    y0 = attn_unit(b, qc, 0)
                    if pending is not None:
                        pb, pqc, pyn = pending
                        osb = oproj_alloc(pb, pqc)
                        oproj_part(pb, pqc, pyn, (0, 1), osb)
                    y1 = attn_unit(b, qc, 1)
                    if pending is not None:
                        oproj_part(pb, pqc, pyn, (2, 3), osb)
                        oproj_flush(pb, pqc, osb)
                    pending = (b, qc, [y0, y1])

                # fully pipelined schedule: every attention unit runs in a
                # projection window as soon as causally possible, so the
                # exp/evac load always has proj matmuls to hide under and
                # there is no attention-only tail
                b0_units = {0: [], 1: [(0, 0)], 2: [(0, 1), (0, 2), (0, 3)],
                            3: [(0, 4), (0, 5)], 4: [(0, 6)]}
                b1_units = {0: [(0, 7), (1, 0), (1, 1)], 1: [(1, 2), (1, 3)],
                            2: [(1, 4), (1, 5)], 3: [(1, 6), (1, 7)]}
                for pi in range(len(chunks[0])):
                    proj_chunk(0, pi)
                    if pi + 1 < len(chunks[0]):
                        if pi + 1 >= 2:
                            prefetch_xt(0, pi + 1)
                    else:
                        prefetch_xt(1, 0)
                    rope_piece(0, pi)
                    for ub, uqc in b0_units[pi]:
                        attn_step(ub, uqc)
                for ci in range(len(chunks[1])):
                    proj_chunk(1, ci)
                    if ci + 1 < len(chunks[1]):
                        prefetch_xt(1, ci + 1)
                    rope_piece(1, ci)
                    for ub, uqc in b1_units[ci]:
                        attn_step(ub, uqc)
                pb, pqc, pyn = pending
                osb = oproj_alloc(pb, pqc)
                oproj_part(pb, pqc, pyn, (0, 1), osb)
                oproj_part(pb, pqc, pyn, (2, 3), osb)
                oproj_flush(pb, pqc, osb)
    nc.finalize()
    return nc


_NC_CACHE = None


def _get_program():
    global _NC_CACHE
    if _NC_CACHE is None:
        _NC_CACHE = _build_program()
    return _NC_CACHE


def _prep_in_maps(x, rotary, qkv_weight, o_weight):
    import jax
    import ml_dtypes

    bf = np.float16
    cpu = jax.devices("cpu")[0]
    with jax.default_device(cpu):
        import jax.numpy as jnp

        sq = jnp.mean(jnp.abs(jnp.asarray(qkv_weight)))
        wq_q = np.asarray(jnp.round(jnp.asarray(qkv_weight) / (sq + EPS)), np.float32)
        so = jnp.mean(jnp.abs(jnp.asarray(o_weight)))
        wo_q = np.asarray(jnp.round(jnp.asarray(o_weight) / (so + EPS)), np.float32)
        sq = float(sq)
        so = float(so)

    xt = np.ascontiguousarray(x.transpose(0, 2, 1)).astype(bf)
    cos_t = np.ascontiguousarray(rotary[1].T).astype(bf)
    sin_t = np.ascontiguousarray(rotary[0].T).astype(np.float32)
    sin_s = sin_t.copy()
    sin_s[:64] *= -1.0
    sin_s = sin_s.astype(bf)

    # aux: mask pair for the diagonal k-tile pair, identity, ones
    kk = np.arange(P)[:, None]
    qq = np.arange(QC)[None, :]
    aux = np.zeros((P, 832), np.float32)
    aux[:, 0:QC] = np.where(qq < kk, MASKV, 0.0)          # B0: k-tile 2qc
    aux[:, QC : 2 * QC] = np.where(qq < kk + P, MASKV, 0.0)  # B1: k-tile 2qc+1
    aux[:, 512:640] = np.eye(P)
    aux[:, 640:768] = 1.0
    aux[:, 768] = -8.0
    aux = aux.astype(bf)

    # fp16 scaling: sqrt(sm_scale) on BOTH q and k weights (scores land fully
    # scaled in PSUM, masks are in post-scale units); final o-scale folded
    # into v weights (keeps every fp16 tensor in normal range; o_weight stays
    # exactly ternary in fp16).
    alpha = np.float32(math.sqrt(sq * sq / math.sqrt(HEAD_DIM)))
    final_scale = np.float32(sq * so)

    in_maps = []
    for c in range(NCORES):
        rows = []
        for part in range(3):  # q, k, v blocks of qkv_weight
            for hl in range(HPC):
                g = HPC * c + hl
                blk = wq_q[part * H + g * HEAD_DIM : part * H + (g + 1) * HEAD_DIM]
                if part < 2:
                    blk = blk * alpha
                else:
                    blk = blk * final_scale
                rows.append(blk)
        wqkv_c = np.ascontiguousarray(np.concatenate(rows, axis=0).T).astype(bf)
        wo_c = np.ascontiguousarray(
            wo_q[:, c * FPC // 3 : (c + 1) * FPC // 3].T
        ).astype(bf)
        in_maps.append(
            {
                "xt": xt,
                "wqkv": wqkv_c,
                "wo": wo_c,
                "cos_t": cos_t,
                "sin_s": sin_s,
                "aux": aux,
            }
        )
    return in_maps


def kernel(x, rotary, qkv_weight, o_weight):
    from concourse.bass_utils import run_bass_kernel_spmd

    in_maps = _prep_in_maps(x, rotary, qkv_weight, o_weight)
    nc = _get_program()
    res = run_bass_kernel_spmd(nc, in_maps, core_ids=list(range(NCORES)))
    acc = res.results[0]["out"].astype(np.float32)
    for c in range(1, NCORES):
        acc = acc + res.results[c]["out"].astype(np.float32)
    return acc


# revision 63
# speedup vs baseline: 1.0263x; 1.0044x over previous
"""Megatron-style tensor-parallel causal attention (BitLinear qkv/o) on 8 TRN2 cores.

Sharding: each core owns 2 of 16 heads (qkv_weight rows) and the matching
256 o_weight columns. x/rotary replicated; partial outputs summed on host.

All SBUF data is f16 (halves DMA + enables DVE 2x modes); PSUM stays fp32.
Causal mask is folded into the score PSUM via an identity-lhsT matmul adding
-60 to masked entries before exp. Softmax denominator uses the all-ones
lhsT matmul (broadcast rows), normalization on DVE before the out-proj.
Emission order proj(b0), proj(b1), attn(b0), attn(b1) keeps the PE busy:
RoPE(b0) on DVE overlaps proj(b1) matmuls, attention overlaps nothing it
needs.
"""

import math

import numpy as np

EPS = 1e-5
NUM_HEADS = 16
HEAD_DIM = 128
B, S, H = 2, 2048, 2048
NCORES = 8
HPC = NUM_HEADS // NCORES        # heads per core = 2
FPC = 3 * HPC * HEAD_DIM         # qkv features per core = 768
P = 128
NHT = H // P                     # 16 h_in tiles
CH = 512                         # proj token chunk
NCH = S // CH                    # 4 chunks per batch
QC = 256                         # attention q chunk
NQC = S // QC                    # 8
MASKV = -60.0


def _build_program():
    import concourse.bacc as bacc
    import concourse.mybir as mybir
    import concourse.tile as tile

    f32 = mybir.dt.float32
    f16 = mybir.dt.float16
    AF = mybir.ActivationFunctionType

    nc = bacc.Bacc(None, target_bir_lowering=False)

    xt = nc.dram_tensor("xt", [B, H, S], f16, kind="ExternalInput")
    wqkv = nc.dram_tensor("wqkv", [H, FPC], f16, kind="ExternalInput")
    wo = nc.dram_tensor("wo", [HPC * HEAD_DIM, H], f16, kind="ExternalInput")
    cos_t = nc.dram_tensor("cos_t", [P, S], f16, kind="ExternalInput")
    sin_s = nc.dram_tensor("sin_s", [P, S], f16, kind="ExternalInput")
    # aux: [0:512) mask pair (B0|B1), [512:640) identity, [640:768) ones
    aux = nc.dram_tensor("aux", [P, 832], f16, kind="ExternalInput")
    out = nc.dram_tensor("out", [B, S, H], f16, kind="ExternalOutput")

    with tile.TileContext(nc) as tc:
        with tc.tile_pool(name="const", bufs=1) as cpool:
            # first proj chunk's x and the first weight slice lead the DMA
            # queue so the PE starts ~9us in instead of ~24us.
            w_sb = cpool.tile([P, NHT, FPC], f16)
            wre = wqkv.rearrange("(t p) f -> p t f", p=P)
            nc.sync.dma_start(w_sb[:, 0:4, :], wre[:, 0:4, :])

            with (
                tc.tile_pool(name="qk", bufs=2) as qkpool,
                tc.tile_pool(name="vv", bufs=2) as vpool,
                tc.tile_pool(name="work", bufs=2) as wpool,
                tc.tile_pool(name="attn", bufs=3) as apool,
                tc.tile_pool(name="outp", bufs=3) as opool,
                tc.psum_pool(name="pproj", bufs=2) as pps,
                tc.psum_pool(name="pop", bufs=2) as opps,
            ):
                # batch-0 leads with two small 256-token chunks so the first
                # proj matmuls start while weights are still streaming in
                chunks = {
                    0: [(0, 256), (256, 256), (512, 512), (1024, 512), (1536, 512)],
                    1: [(c * CH, CH) for c in range(NCH)],
                }
                # startup: interleave w and first-bite x by h-group
                xt0 = wpool.tile([P, NHT, 256], f16, tag="xt")
                xre0 = xt[0, :, 0:256].rearrange("(t p) c -> p t c", p=P)
                nc.sync.dma_start(xt0[:, 0:4, :], xre0[:, 0:4, :])
                for hgrp in range(1, 4):
                    nc.sync.dma_start(
                        w_sb[:, 4 * hgrp : 4 * (hgrp + 1), :],
                        wre[:, 4 * hgrp : 4 * (hgrp + 1), :],
                    )
                    nc.sync.dma_start(
                        xt0[:, 4 * hgrp : 4 * (hgrp + 1), :],
                        xre0[:, 4 * hgrp : 4 * (hgrp + 1), :],
                    )
                # second bite before the (later-needed) constants
                xt1 = wpool.tile([P, NHT, 256], f16, tag="xt")
                nc.sync.dma_start(
                    xt1[:], xt[0, :, 256:512].rearrange("(t p) c -> p t c", p=P)
                )
                wo_sb = cpool.tile([P, HPC, H], f16)
                nc.sync.dma_start(wo_sb[:], wo.rearrange("(t p) o -> p t o", p=P))
                aux_sb = cpool.tile([P, 832], f16)
                nc.sync.dma_start(aux_sb[:], aux[:])
                rot_sb = cpool.tile([P, 2 * S], f16)
                nc.sync.dma_start(rot_sb[:, 0:S], cos_t[:])
                nc.sync.dma_start(rot_sb[:, S : 2 * S], sin_s[:])

                msk = aux_sb[:, 0:512]          # [k,128] x (B0|B1) for diag pair
                iden = aux_sb[:, 512:640]       # identity
                ones = aux_sb[:, 640:768]       # all-ones
                expb = aux_sb[:, 768:769]       # exp bias column (-8)

                qk_raw = {}   # (b, f) -> raw (pre-rope) tiles
                qk_rope = {}  # (b, f) -> roped tiles
                v_sb = {}     # b -> v tiles [tok_part, ktile, hl*128]
                for b in range(B):
                    for f in range(4):
                        qk_raw[b, f] = qkpool.tile(
                            [P, S], f16, tag=f"qkr{f}", name=f"qkr{f}_{b}"
                        )
                        qk_rope[b, f] = qkpool.tile(
                            [P, S], f16, tag=f"qkf{f}", name=f"qkf{f}_{b}"
                        )
                    v_sb[b] = vpool.tile(
                        [P, (S // P) * 2 * P], f16, tag="v", name=f"v_{b}"
                    )

                # ---------------- projection (+rope) -----------------------
                xt_pre = {}

                def prefetch_xt(b, ci):
                    t0c, W = chunks[b][ci]
                    t = wpool.tile(
                        [P, NHT, W], f16, tag="xt", name=f"xt_{b}_{ci}"
                    )
                    nc.sync.dma_start(
                        t[:],
                        xt[b, :, t0c : t0c + W].rearrange("(t p) c -> p t c", p=P),
                    )
                    xt_pre[b, ci] = t

                def proj_chunk(b, ci):
                    t0c, W = chunks[b][ci]
                    if True:
                        if b == 0 and ci == 0:
                            xt_sb = xt0
                        elif b == 0 and ci == 1:
                            xt_sb = xt1
                        elif (b, ci) in xt_pre:
                            xt_sb = xt_pre.pop((b, ci))
                        else:
                            xt_sb = wpool.tile(
                                [P, NHT, W], f16,
                                tag="xt", name=f"xt_{b}_{ci}",
                            )
                            nc.sync.dma_start(
                                xt_sb[:],
                                xt[b, :, t0c : t0c + W].rearrange(
                                    "(t p) c -> p t c", p=P
                                ),
                            )
                        # q0,q1,k0,k1 : [feat, tok]
                        for f in range(4):
                            ps = pps.tile([P, W], f32, tag="proj", name=f"ps{b}_{ci}_{f}")
                            for h in range(NHT):
                                nc.tensor.matmul(
                                    ps[:],
                                    lhsT=w_sb[:, h, f * P : (f + 1) * P],
                                    rhs=xt_sb[:, h, :],
                                    start=(h == 0),
                                    stop=(h == NHT - 1),
                                )
                            if f % 2 == 0:
                                nc.scalar.copy(
                                    qk_raw[b, f][:, t0c : t0c + W], ps[:]
                                )
                            else:
                                nc.vector.tensor_copy(
                                    qk_raw[b, f][:, t0c : t0c + W], ps[:]
                                )
                        # v: [tok, feat] two tok-subs per psum tile
                        for half in range(W // 256):
                            psv = pps.tile(
                                [P, 512], f32, tag="proj", name=f"psv{b}_{ci}_{half}"
                            )
                            for sub in range(2):
                                tsub = half * 2 + sub
                                for h in range(NHT):
                                    nc.tensor.matmul(
                                        psv[:, sub * 2 * P : (sub + 1) * 2 * P],
                                        lhsT=xt_sb[:, h, tsub * P : (tsub + 1) * P],
                                        rhs=w_sb[:, h, 4 * P : 6 * P],
                                        start=(h == 0),
                                        stop=(h == NHT - 1),
                                    )
                            kt0 = t0c // P + half * 2
                            if half == 0:
                                nc.scalar.copy(
                                    v_sb[b][:, kt0 * 2 * P : (kt0 + 2) * 2 * P], psv[:]
                                )
                            else:
                                nc.vector.tensor_copy(
                                    v_sb[b][:, kt0 * 2 * P : (kt0 + 2) * 2 * P], psv[:]
                                )
                def rope_piece(b, pi):
                    # rope one proj chunk's span; runs on DVE under the next
                    # chunk's proj matmuls
                    t0c, W = chunks[b][pi]
                    for f in range(4):
                        raw = qk_raw[b, f]
                        qsw = wpool.tile(
                            [P, W], f16, tag="qsw", name=f"qsw{b}_{pi}_{f}"
                        )
                        nc.sync.dma_start(
                            qsw[0:64, :], raw[64:128, t0c : t0c + W]
                        )
                        nc.sync.dma_start(
                            qsw[64:128, :], raw[0:64, t0c : t0c + W]
                        )
                        m1 = wpool.tile(
                            [P, W], f16, tag="m1", name=f"m1{b}_{pi}_{f}"
                        )
                        nc.vector.tensor_mul(
                            m1[:], raw[:, t0c : t0c + W], rot_sb[:, t0c : t0c + W]
                        )
                        nc.vector.tensor_mul(
                            qsw[:], qsw[:], rot_sb[:, S + t0c : S + t0c + W]
                        )
                        nc.vector.tensor_add(
                            qk_rope[b, f][:, t0c : t0c + W], m1[:], qsw[:]
                        )

                # ---------------- attention + out-proj ----------------------
                # The last k-tile of each q-chunk only covers q[128:256)
                # (ragged trim). Denominator: full pairs are pre-summed on DVE
                # (halves the ones-matmul rows); the ones-matmul for pair g is
                # deferred until after pair g+1's attn*v so the PE never waits
                # on the DVE add.
                def attn_unit(b, qc, hl):
                    q_t = qk_rope[b, hl]
                    k_t = qk_rope[b, 2 + hl]
                    qs = q_t[:, qc * QC : (qc + 1) * QC]
                    qs_hi = q_t[:, qc * QC + P : (qc + 1) * QC]
                    yt = opps.tile([P, 512], f32, tag="op", name=f"yt{b}_{qc}_{hl}", bufs=5)
                    sm = opps.tile([P, QC], f32, tag="sum", name=f"sm{b}_{qc}_{hl}", bufs=1)
                    pend = None       # deferred exs tile for the ones-matmul
                    sum_started = False

                    def ones_mm(rhs_ap, region, stop):
                        nonlocal sum_started
                        nc.tensor.matmul(
                            sm[:, region[0] : region[1]],
                            lhsT=ones,
                            rhs=rhs_ap,
                            start=not sum_started,
                            stop=stop,
                        )
                        sum_started = True

                    def emit_scores(g):
                        diag = g == qc
                        sc = opps.tile(
                            [P, 2 * QC], f32, tag="op", bufs=5,
                            name=f"sc{b}_{qc}_{hl}_{g}",
                        )
                        nc.tensor.matmul(
                            sc[:, 0:QC],
                            lhsT=k_t[:, 2 * g * P : (2 * g + 1) * P],
                            rhs=qs,
                            start=True,
                            stop=not diag,
                        )
                        if diag:
                            # only the left [128,128] of this tile is masked
                            nc.tensor.matmul(
                                sc[:, 0:P], lhsT=iden, rhs=msk[:, 0:P],
                                start=False, stop=True,
                            )
                            nc.tensor.matmul(
                                sc[:, QC : QC + P],
                                lhsT=k_t[:, (2 * g + 1) * P : (2 * g + 2) * P],
                                rhs=qs_hi,
                                start=True,
                                stop=False,
                            )
                            nc.tensor.matmul(
                                sc[:, QC : QC + P], lhsT=iden, rhs=msk[:, 0:P],
                                start=False, stop=True,
                            )
                        else:
                            nc.tensor.matmul(
                                sc[:, QC : 2 * QC],
                                lhsT=k_t[:, (2 * g + 1) * P : (2 * g + 2) * P],
                                rhs=qs,
                                start=True,
                                stop=True,
                            )
                        return sc

                    # 3-stage pipeline: scores(g+2) and exp(g+1) run ahead of
                    # attn*v(g), so the PE never waits on the Activation
                    # engine's exp. Denominator adds (DVE) get a full
                    # iteration of slack before their ones-matmul.
                    exd = {}   # g -> (ex tile, exs tile or None)

                    def emit_exp(g):
                        diag = g == qc
                        scw = 2 * QC if not diag else QC + P
                        ex = apool.tile([P, scw], f16, tag="ex")
                        nc.scalar.activation(
                            ex[:], scd[g][:, 0:scw], AF.Exp, bias=expb
                        )
                        if not diag:
                            exs = apool.tile([P, QC], f16, tag="exs", bufs=4)
                            nc.vector.tensor_add(
                                exs[:], ex[:, 0:QC], ex[:, QC : 2 * QC]
                            )
                        else:
                            # combine the two k-tiles' shared q-half so the
                            # diagonal denominator is two 128-row matmuls
                            exs = apool.tile([P, P], f16, tag="exs", bufs=4)
                            nc.vector.tensor_add(
                                exs[:], ex[:, P:QC], ex[:, QC : QC + P]
                            )
                        exd[g] = (ex, exs)

                    def emit_av(g):
                        nonlocal qpend, ppend, opend
                        diag = g == qc
                        scw = 2 * QC if not diag else QC + P
                        ex, exs = exd.pop(g)
                        v0 = 2 * g * 2 * P + hl * P
                        nc.tensor.matmul(
                            yt[:, 0:QC],
                            lhsT=v_sb[b][:, v0 : v0 + P],
                            rhs=ex[:, 0:QC],
                            start=(g == 0),
                            stop=False,
                        )
                        v1 = (2 * g + 1) * 2 * P + hl * P
                        nc.tensor.matmul(
                            yt[:, P:QC] if diag else yt[:, 0:QC],
                            lhsT=v_sb[b][:, v1 : v1 + P],
                            rhs=ex[:, QC:scw],
                            start=False,
                            stop=diag,
                        )
                        # quad-summed denominator: ones-matmuls run on
                        # pair-of-pair sums, each deferred one iteration so
                        # the PE never waits on the DVE adds
                        if opend is not None:
                            ones_mm(opend[:], (0, QC), stop=False)
                            opend = None
                        if not diag:
                            if qpend is None:
                                qpend = exs
                            else:
                                exq = apool.tile(
                                    [P, QC], f16, tag="exq",
                                    name=f"exq{b}_{qc}_{hl}_{g}", bufs=5,
                                )
                                nc.vector.tensor_add(exq[:], qpend[:], exs[:])
                                qpend = None
                                if ppend is None:
                                    ppend = exq
                                else:
                                    exo = apool.tile(
                                        [P, QC], f16, tag="exo",
                                        name=f"exo{b}_{qc}_{hl}_{g}",
                                    )
                                    nc.vector.tensor_add(
                                        exo[:], ppend[:], exq[:]
                                    )
                                    ppend = None
                                    opend = exo
                        elif qc == 0:
                            # no prior pair zeroed the region: cover all of it
                            ones_mm(ex[:, 0:QC], (0, QC), stop=False)
                            ones_mm(ex[:, QC : QC + P], (P, QC), stop=True)
                        else:
                            if ppend is not None:
                                ones_mm(ppend[:], (0, QC), stop=False)
                                ppend = None
                            if qpend is not None:
                                ones_mm(qpend[:], (0, QC), stop=False)
                                qpend = None
                            ones_mm(ex[:, 0:P], (0, P), stop=False)
                            ones_mm(exs[:], (P, QC), stop=True)

                    qpend = None   # pair sum awaiting its quad partner
                    ppend = None   # quad sum awaiting its octet partner
                    opend = None   # tree sum awaiting its ones-matmul
                    scd = {0: emit_scores(0)}
                    if qc >= 1:
                        scd[1] = emit_scores(1)
                    for g in range(qc + 1):
                        emit_exp(g)
                        if g + 2 <= qc:
                            scd[g + 2] = emit_scores(g + 2)
                        if g >= 1:
                            emit_av(g - 1)
                    emit_av(qc)
                    recip = apool.tile([P, QC], f32, tag="rc")
                    nc.vector.reciprocal(recip[:], sm[:])
                    y = apool.tile([P, QC], f16, tag=f"yn{hl}")
                    nc.vector.tensor_mul(y[:], yt[:, 0:QC], recip[:])
                    return y

                def oproj_part(b, qc, yn, quarters, os_sb):
                    # sub-interleaved so each quarter's PSUM drain overlaps
                    # the other sub's matmuls
                    for quarter in quarters:
                        for sub in range(2):
                            ops = opps.tile([P, 512], f32, tag="op", bufs=5)
                            for hl in range(2):
                                nc.tensor.matmul(
                                    ops[:],
                                    lhsT=yn[hl][:, sub * P : (sub + 1) * P],
                                    rhs=wo_sb[:, hl, quarter * 512 : (quarter + 1) * 512],
                                    start=(hl == 0),
                                    stop=(hl == 1),
                                )
                            if (quarter * 2 + sub) % 2 == 0:
                                nc.scalar.copy(
                                    os_sb[sub][:, quarter * 512 : (quarter + 1) * 512],
                                    ops[:],
                                )
                            else:
                                nc.vector.tensor_copy(
                                    os_sb[sub][:, quarter * 512 : (quarter + 1) * 512],
                                    ops[:],
                                )

                def oproj_alloc(b, qc):
                    return [
                        opool.tile([P, H], f16, tag="os", name=f"os{b}_{qc}_{s}")
                        for s in range(2)
                    ]

                def oproj_flush(b, qc, os_sb):
                    for sub in range(2):
                        t0 = qc * QC + sub * P
                        nc.sync.dma_start(out[b, t0 : t0 + P, :], os_sb[sub][:])

                pending = None

                def attn_step(b, qc):
                    # previous chunk's out-proj lands in two half-bursts
                    # around this chunk's second head-unit: PE work that
                    # hides the normalize chain and spreads evacuations
                    nonlocal pending
                    y0 = attn_unit(b, qc, 0)
                    if pending is not None:
                        pb, pqc, pyn = pending
                        osb = oproj_alloc(pb, pqc)
                        oproj_part(pb, pqc, pyn, (0, 1), osb)
                    y1 = attn_unit(b, qc, 1)
                    if pending is not None:
                        oproj_part(pb, pqc, pyn, (2, 3), osb)
                        oproj_flush(pb, pqc, osb)
                    pending = (b, qc, [y0, y1])

                # fully pipelined schedule: every attention unit runs in a
                # projection window as soon as causally possible, so the
                # exp/evac load always has proj matmuls to hide under and
                # there is no attention-only tail
                b0_units = {0: [], 1: [(0, 0)], 2: [(0, 1), (0, 2), (0, 3)],
                            3: [(0, 4), (0, 5)], 4: [(0, 6)]}
                b1_units = {0: [(0, 7), (1, 0), (1, 1)], 1: [(1, 2), (1, 3)],
                            2: [(1, 4), (1, 5)], 3: [(1, 6), (1, 7)]}
                for pi in range(len(chunks[0])):
                    proj_chunk(0, pi)
                    if pi + 1 < len(chunks[0]):
                        if pi + 1 >= 2:
                            prefetch_xt(0, pi + 1)
                    else:
                        prefetch_xt(1, 0)
                    rope_piece(0, pi)
                    for ub, uqc in b0_units[pi]:
                        attn_step(ub, uqc)
                for ci in range(len(chunks[1])):
                    proj_chunk(1, ci)
                    if ci + 1 < len(chunks[1]):
                        prefetch_xt(1, ci + 1)
                    rope_piece(1, ci)
                    for ub, uqc in b1_units[ci]:
                        attn_step(ub, uqc)
                pb, pqc, pyn = pending
                osb = oproj_alloc(pb, pqc)
                oproj_part(pb, pqc, pyn, (0, 1), osb)
                oproj_part(pb, pqc, pyn, (2, 3), osb)
                oproj_flush(pb, pqc, osb)
    nc.finalize()
    return nc


_NC_CACHE = None


def _get_program():
    global _NC_CACHE
    if _NC_CACHE is None:
        _NC_CACHE = _build_program()
    return _NC_CACHE


def _prep_in_maps(x, rotary, qkv_weight, o_weight):
    import jax
    import ml_dtypes

    bf = np.float16
    cpu = jax.devices("cpu")[0]
    with jax.default_device(cpu):
        import jax.numpy as jnp

        sq = jnp.mean(jnp.abs(jnp.asarray(qkv_weight)))
        wq_q = np.asarray(jnp.round(jnp.asarray(qkv_weight) / (sq + EPS)), np.float32)
        so = jnp.mean(jnp.abs(jnp.asarray(o_weight)))
        wo_q = np.asarray(jnp.round(jnp.asarray(o_weight) / (so + EPS)), np.float32)
        sq = float(sq)
        so = float(so)

    xt = np.ascontiguousarray(x.transpose(0, 2, 1)).astype(bf)
    cos_t = np.ascontiguousarray(rotary[1].T).astype(bf)
    sin_t = np.ascontiguousarray(rotary[0].T).astype(np.float32)
    sin_s = sin_t.copy()
    sin_s[:64] *= -1.0
    sin_s = sin_s.astype(bf)

    # aux: mask pair for the diagonal k-tile pair, identity, ones
    kk = np.arange(P)[:, None]
    qq = np.arange(QC)[None, :]
    aux = np.zeros((P, 832), np.float32)
    aux[:, 0:QC] = np.where(qq < kk, MASKV, 0.0)          # B0: k-tile 2qc
    aux[:, QC : 2 * QC] = np.where(qq < kk + P, MASKV, 0.0)  # B1: k-tile 2qc+1
    aux[:, 512:640] = np.eye(P)
    aux[:, 640:768] = 1.0
    aux[:, 768] = -8.0
    aux = aux.astype(bf)

    # fp16 scaling: sqrt(sm_scale) on BOTH q and k weights (scores land fully
    # scaled in PSUM, masks are in post-scale units); final o-scale folded
    # into v weights (keeps every fp16 tensor in normal range; o_weight stays
    # exactly ternary in fp16).
    alpha = np.float32(math.sqrt(sq * sq / math.sqrt(HEAD_DIM)))
    final_scale = np.float32(sq * so)

    in_maps = []
    for c in range(NCORES):
        rows = []
        for part in range(3):  # q, k, v blocks of qkv_weight
            for hl in range(HPC):
                g = HPC * c + hl
                blk = wq_q[part * H + g * HEAD_DIM : part * H + (g + 1) * HEAD_DIM]
                if part < 2:
                    blk = blk * alpha
                else:
                    blk = blk * final_scale
                rows.append(blk)
        wqkv_c = np.ascontiguousarray(np.concatenate(rows, axis=0).T).astype(bf)
        wo_c = np.ascontiguousarray(
            wo_q[:, c * FPC // 3 : (c + 1) * FPC // 3].T
        ).astype(bf)
        in_maps.append(
            {
                "xt": xt,
                "wqkv": wqkv_c,
                "wo": wo_c,
                "cos_t": cos_t,
                "sin_s": sin_s,
                "aux": aux,
            }
        )
    return in_maps


def kernel(x, rotary, qkv_weight, o_weight):
    from concourse.bass_utils import run_bass_kernel_spmd

    in_maps = _prep_in_maps(x, rotary, qkv_weight, o_weight)
    nc = _get_program()
    res = run_bass_kernel_spmd(nc, in_maps, core_ids=list(range(NCORES)))
    acc = res.results[0]["out"].astype(np.float32)
    for c in range(1, NCORES):
        acc = acc + res.results[c]["out"].astype(np.float32)
    return acc


# revision 64
# speedup vs baseline: 1.0329x; 1.0064x over previous
"""Megatron-style tensor-parallel causal attention (BitLinear qkv/o) on 8 TRN2 cores.

Sharding: each core owns 2 of 16 heads (qkv_weight rows) and the matching
256 o_weight columns. x/rotary replicated; partial outputs summed on host.

All SBUF data is f16 (halves DMA + enables DVE 2x modes); PSUM stays fp32.
Causal mask is folded into the score PSUM via an identity-lhsT matmul adding
-60 to masked entries before exp. Softmax denominator uses the all-ones
lhsT matmul (broadcast rows), normalization on DVE before the out-proj.
Emission order proj(b0), proj(b1), attn(b0), attn(b1) keeps the PE busy:
RoPE(b0) on DVE overlaps proj(b1) matmuls, attention overlaps nothing it
needs.
"""

import math

import numpy as np

EPS = 1e-5
NUM_HEADS = 16
HEAD_DIM = 128
B, S, H = 2, 2048, 2048
NCORES = 8
HPC = NUM_HEADS // NCORES        # heads per core = 2
FPC = 3 * HPC * HEAD_DIM         # qkv features per core = 768
P = 128
NHT = H // P                     # 16 h_in tiles
CH = 512                         # proj token chunk
NCH = S // CH                    # 4 chunks per batch
QC = 256                         # attention q chunk
NQC = S // QC                    # 8
MASKV = -60.0


def _build_program():
    import concourse.bacc as bacc
    import concourse.mybir as mybir
    import concourse.tile as tile

    f32 = mybir.dt.float32
    f16 = mybir.dt.float16
    AF = mybir.ActivationFunctionType

    nc = bacc.Bacc(None, target_bir_lowering=False)

    xt = nc.dram_tensor("xt", [B, H, S], f16, kind="ExternalInput")
    wqkv = nc.dram_tensor("wqkv", [H, FPC], f16, kind="ExternalInput")
    wo = nc.dram_tensor("wo", [HPC * HEAD_DIM, H], f16, kind="ExternalInput")
    cos_t = nc.dram_tensor("cos_t", [P, S], f16, kind="ExternalInput")
    sin_s = nc.dram_tensor("sin_s", [P, S], f16, kind="ExternalInput")
    # aux: [0:512) mask pair (B0|B1), [512:640) identity, [640:768) ones
    aux = nc.dram_tensor("aux", [P, 832], f16, kind="ExternalInput")
    out = nc.dram_tensor("out", [B, S, H], f16, kind="ExternalOutput")

    with tile.TileContext(nc) as tc:
        with tc.tile_pool(name="const", bufs=1) as cpool:
            # first proj chunk's x and the first weight slice lead the DMA
            # queue so the PE starts ~9us in instead of ~24us.
            w_sb = cpool.tile([P, NHT, FPC], f16)
            wre = wqkv.rearrange("(t p) f -> p t f", p=P)
            nc.sync.dma_start(w_sb[:, 0:4, :], wre[:, 0:4, :])

            with (
                tc.tile_pool(name="qk", bufs=2) as qkpool,
                tc.tile_pool(name="vv", bufs=2) as vpool,
                tc.tile_pool(name="work", bufs=2) as wpool,
                tc.tile_pool(name="attn", bufs=3) as apool,
                tc.tile_pool(name="outp", bufs=3) as opool,
                tc.psum_pool(name="pproj", bufs=2) as pps,
                tc.psum_pool(name="pop", bufs=2) as opps,
            ):
                # batch-0 leads with two small 256-token chunks so the first
                # proj matmuls start while weights are still streaming in
                chunks = {
                    0: [(0, 256), (256, 256), (512, 512), (1024, 512), (1536, 512)],
                    1: [(c * CH, CH) for c in range(NCH)],
                }
                # startup: interleave w and first-bite x by h-group
                xt0 = wpool.tile([P, NHT, 256], f16, tag="xt")
                xre0 = xt[0, :, 0:256].rearrange("(t p) c -> p t c", p=P)
                nc.sync.dma_start(xt0[:, 0:4, :], xre0[:, 0:4, :])
                for hgrp in range(1, 4):
                    nc.sync.dma_start(
                        w_sb[:, 4 * hgrp : 4 * (hgrp + 1), :],
                        wre[:, 4 * hgrp : 4 * (hgrp + 1), :],
                    )
                    nc.sync.dma_start(
                        xt0[:, 4 * hgrp : 4 * (hgrp + 1), :],
                        xre0[:, 4 * hgrp : 4 * (hgrp + 1), :],
                    )
                # second bite before the (later-needed) constants
                xt1 = wpool.tile([P, NHT, 256], f16, tag="xt")
                nc.sync.dma_start(
                    xt1[:], xt[0, :, 256:512].rearrange("(t p) c -> p t c", p=P)
                )
                wo_sb = cpool.tile([P, HPC, H], f16)
                nc.sync.dma_start(wo_sb[:], wo.rearrange("(t p) o -> p t o", p=P))
                aux_sb = cpool.tile([P, 832], f16)
                nc.sync.dma_start(aux_sb[:], aux[:])
                rot_sb = cpool.tile([P, 2 * S], f16)
                nc.sync.dma_start(rot_sb[:, 0:S], cos_t[:])
                nc.sync.dma_start(rot_sb[:, S : 2 * S], sin_s[:])

                msk = aux_sb[:, 0:512]          # [k,128] x (B0|B1) for diag pair
                iden = aux_sb[:, 512:640]       # identity
                ones = aux_sb[:, 640:768]       # all-ones
                expb = aux_sb[:, 768:769]       # exp bias column (-8)

                qk_raw = {}   # (b, f) -> raw (pre-rope) tiles
                qk_rope = {}  # (b, f) -> roped tiles
                v_sb = {}     # b -> v tiles [tok_part, ktile, hl*128]
                for b in range(B):
                    for f in range(4):
                        qk_raw[b, f] = qkpool.tile(
                            [P, S], f16, tag=f"qkr{f}", name=f"qkr{f}_{b}"
                        )
                        qk_rope[b, f] = qkpool.tile(
                            [P, S], f16, tag=f"qkf{f}", name=f"qkf{f}_{b}"
                        )
                    v_sb[b] = vpool.tile(
                        [P, (S // P) * 2 * P], f16, tag="v", name=f"v_{b}"
                    )

                # ---------------- projection (+rope) -----------------------
                xt_pre = {}

                def prefetch_xt(b, ci):
                    t0c, W = chunks[b][ci]
                    t = wpool.tile(
                        [P, NHT, W], f16, tag="xt", name=f"xt_{b}_{ci}"
                    )
                    nc.sync.dma_start(
                        t[:],
                        xt[b, :, t0c : t0c + W].rearrange("(t p) c -> p t c", p=P),
                    )
                    xt_pre[b, ci] = t

                def proj_chunk(b, ci):
                    t0c, W = chunks[b][ci]
                    if True:
                        if b == 0 and ci == 0:
                            xt_sb = xt0
                        elif b == 0 and ci == 1:
                            xt_sb = xt1
                        elif (b, ci) in xt_pre:
                            xt_sb = xt_pre.pop((b, ci))
                        else:
                            xt_sb = wpool.tile(
                                [P, NHT, W], f16,
                                tag="xt", name=f"xt_{b}_{ci}",
                            )
                            nc.sync.dma_start(
                                xt_sb[:],
                                xt[b, :, t0c : t0c + W].rearrange(
                                    "(t p) c -> p t c", p=P
                                ),
                            )
                        # q0,q1,k0,k1 : [feat, tok]
                        for f in range(4):
                            ps = pps.tile([P, W], f32, tag="proj", name=f"ps{b}_{ci}_{f}")
                            for h in range(NHT):
                                nc.tensor.matmul(
                                    ps[:],
                                    lhsT=w_sb[:, h, f * P : (f + 1) * P],
                                    rhs=xt_sb[:, h, :],
                                    start=(h == 0),
                                    stop=(h == NHT - 1),
                                )
                            if f % 2 == 0:
                                nc.scalar.copy(
                                    qk_raw[b, f][:, t0c : t0c + W], ps[:]
                                )
                            else:
                                nc.vector.tensor_copy(
                                    qk_raw[b, f][:, t0c : t0c + W], ps[:]
                                )
                        # v: [tok, feat] two tok-subs per psum tile
                        for half in range(W // 256):
                            psv = pps.tile(
                                [P, 512], f32, tag="proj", name=f"psv{b}_{ci}_{half}"
                            )
                            for sub in range(2):
                                tsub = half * 2 + sub
                                for h in range(NHT):
                                    nc.tensor.matmul(
                                        psv[:, sub * 2 * P : (sub + 1) * 2 * P],
                                        lhsT=xt_sb[:, h, tsub * P : (tsub + 1) * P],
                                        rhs=w_sb[:, h, 4 * P : 6 * P],
                                        start=(h == 0),
                                        stop=(h == NHT - 1),
                                    )
                            kt0 = t0c // P + half * 2
                            if half == 0:
                                nc.scalar.copy(
                                    v_sb[b][:, kt0 * 2 * P : (kt0 + 2) * 2 * P], psv[:]
                                )
                            else:
                                nc.vector.tensor_copy(
                                    v_sb[b][:, kt0 * 2 * P : (kt0 + 2) * 2 * P], psv[:]
                                )
                def rope_piece(b, pi):
                    # rope one proj chunk's span; runs on DVE under the next
                    # chunk's proj matmuls
                    t0c, W = chunks[b][pi]
                    for f in range(4):
                        raw = qk_raw[b, f]
                        qsw = wpool.tile(
                            [P, W], f16, tag="qsw", name=f"qsw{b}_{pi}_{f}"
                        )
                        nc.sync.dma_start(
                            qsw[0:64, :], raw[64:128, t0c : t0c + W]
                        )
                        nc.sync.dma_start(
                            qsw[64:128, :], raw[0:64, t0c : t0c + W]
                        )
                        m1 = wpool.tile(
                            [P, W], f16, tag="m1", name=f"m1{b}_{pi}_{f}"
                        )
                        nc.vector.tensor_mul(
                            m1[:], raw[:, t0c : t0c + W], rot_sb[:, t0c : t0c + W]
                        )
                        nc.vector.tensor_mul(
                            qsw[:], qsw[:], rot_sb[:, S + t0c : S + t0c + W]
                        )
                        nc.vector.tensor_add(
                            qk_rope[b, f][:, t0c : t0c + W], m1[:], qsw[:]
                        )

                # ---------------- attention + out-proj ----------------------
                # The last k-tile of each q-chunk only covers q[128:256)
                # (ragged trim). Denominator: full pairs are pre-summed on DVE
                # (halves the ones-matmul rows); the ones-matmul for pair g is
                # deferred until after pair g+1's attn*v so the PE never waits
                # on the DVE add.
                def attn_unit(b, qc, hl):
                    q_t = qk_rope[b, hl]
                    k_t = qk_rope[b, 2 + hl]
                    qs = q_t[:, qc * QC : (qc + 1) * QC]
                    qs_hi = q_t[:, qc * QC + P : (qc + 1) * QC]
                    yt = opps.tile([P, 512], f32, tag="op", name=f"yt{b}_{qc}_{hl}", bufs=5)
                    sm = opps.tile([P, QC], f32, tag="sum", name=f"sm{b}_{qc}_{hl}", bufs=1)
                    pend = None       # deferred exs tile for the ones-matmul
                    sum_started = False

                    def ones_mm(rhs_ap, region, stop):
                        nonlocal sum_started
                        nc.tensor.matmul(
                            sm[:, region[0] : region[1]],
                            lhsT=ones,
                            rhs=rhs_ap,
                            start=not sum_started,
                            stop=stop,
                        )
                        sum_started = True

                    def emit_scores(g):
                        diag = g == qc
                        sc = opps.tile(
                            [P, 2 * QC], f32, tag="op", bufs=5,
                            name=f"sc{b}_{qc}_{hl}_{g}",
                        )
                        nc.tensor.matmul(
                            sc[:, 0:QC],
                            lhsT=k_t[:, 2 * g * P : (2 * g + 1) * P],
                            rhs=qs,
                            start=True,
                            stop=not diag,
                        )
                        if diag:
                            # only the left [128,128] of this tile is masked
                            nc.tensor.matmul(
                                sc[:, 0:P], lhsT=iden, rhs=msk[:, 0:P],
                                start=False, stop=True,
                            )
                            nc.tensor.matmul(
                                sc[:, QC : QC + P],
                                lhsT=k_t[:, (2 * g + 1) * P : (2 * g + 2) * P],
                                rhs=qs_hi,
                                start=True,
                                stop=False,
                            )
                            nc.tensor.matmul(
                                sc[:, QC : QC + P], lhsT=iden, rhs=msk[:, 0:P],
                                start=False, stop=True,
                            )
                        else:
                            nc.tensor.matmul(
                                sc[:, QC : 2 * QC],
                                lhsT=k_t[:, (2 * g + 1) * P : (2 * g + 2) * P],
                                rhs=qs,
                                start=True,
                                stop=True,
                            )
                        return sc

                    # 3-stage pipeline: scores(g+2) and exp(g+1) run ahead of
                    # attn*v(g), so the PE never waits on the Activation
                    # engine's exp. Denominator adds (DVE) get a full
                    # iteration of slack before their ones-matmul.
                    exd = {}   # g -> (ex tile, exs tile or None)

                    def emit_exp(g):
                        nonlocal qpend, ppend
                        diag = g == qc
                        scw = 2 * QC if not diag else QC + P
                        ex = apool.tile([P, scw], f16, tag="ex")
                        nc.scalar.activation(
                            ex[:], scd[g][:, 0:scw], AF.Exp, bias=expb
                        )
                        if not diag:
                            exs = apool.tile([P, QC], f16, tag="exs", bufs=4)
                            nc.vector.tensor_add(
                                exs[:], ex[:, 0:QC], ex[:, QC : 2 * QC]
                            )
                        else:
                            # combine the two k-tiles' shared q-half so the
                            # diagonal denominator is two 128-row matmuls
                            exs = apool.tile([P, P], f16, tag="exs", bufs=4)
                            nc.vector.tensor_add(
                                exs[:], ex[:, P:QC], ex[:, QC : QC + P]
                            )
                            # fold the q-sub0 piece into a leftover pending
                            # sum if one exists (saves its ones-matmul)
                            tgt = ppend if ppend is not None else qpend
                            if tgt is not None:
                                nc.vector.tensor_add(
                                    tgt[:, 0:P], tgt[:, 0:P], ex[:, 0:P]
                                )
                                dmerged[0] = True
                        exd[g] = (ex, exs)

                    def emit_av(g):
                        nonlocal qpend, ppend, opend
                        diag = g == qc
                        scw = 2 * QC if not diag else QC + P
                        ex, exs = exd.pop(g)
                        v0 = 2 * g * 2 * P + hl * P
                        nc.tensor.matmul(
                            yt[:, 0:QC],
                            lhsT=v_sb[b][:, v0 : v0 + P],
                            rhs=ex[:, 0:QC],
                            start=(g == 0),
                            stop=False,
                        )
                        v1 = (2 * g + 1) * 2 * P + hl * P
                        nc.tensor.matmul(
                            yt[:, P:QC] if diag else yt[:, 0:QC],
                            lhsT=v_sb[b][:, v1 : v1 + P],
                            rhs=ex[:, QC:scw],
                            start=False,
                            stop=diag,
                        )
                        # quad-summed denominator: ones-matmuls run on
                        # pair-of-pair sums, each deferred one iteration so
                        # the PE never waits on the DVE adds
                        if opend is not None:
                            ones_mm(opend[:], (0, QC), stop=False)
                            opend = None
                        if not diag:
                            if qpend is None:
                                qpend = exs
                            else:
                                exq = apool.tile(
                                    [P, QC], f16, tag="exq",
                                    name=f"exq{b}_{qc}_{hl}_{g}", bufs=5,
                                )
                                nc.vector.tensor_add(exq[:], qpend[:], exs[:])
                                qpend = None
                                if ppend is None:
                                    ppend = exq
                                else:
                                    exo = apool.tile(
                                        [P, QC], f16, tag="exo",
                                        name=f"exo{b}_{qc}_{hl}_{g}",
                                    )
                                    nc.vector.tensor_add(
                                        exo[:], ppend[:], exq[:]
                                    )
                                    ppend = None
                                    opend = exo
                        elif qc == 0:
                            # no prior pair zeroed the region: cover all of it
                            ones_mm(ex[:, 0:QC], (0, QC), stop=False)
                            ones_mm(ex[:, QC : QC + P], (P, QC), stop=True)
                        else:
                            if ppend is not None:
                                ones_mm(ppend[:], (0, QC), stop=False)
                                ppend = None
                            if qpend is not None:
                                ones_mm(qpend[:], (0, QC), stop=False)
                                qpend = None
                            if not dmerged[0]:
                                ones_mm(ex[:, 0:P], (0, P), stop=False)
                            ones_mm(exs[:], (P, QC), stop=True)

                    dmerged = [False]  # diag piece folded into a pending sum
                    qpend = None   # pair sum awaiting its quad partner
                    ppend = None   # quad sum awaiting its octet partner
                    opend = None   # tree sum awaiting its ones-matmul
                    scd = {0: emit_scores(0)}
                    if qc >= 1:
                        scd[1] = emit_scores(1)
                    for g in range(qc):
                        emit_exp(g)
                        if g + 2 <= qc:
                            scd[g + 2] = emit_scores(g + 2)
                        if g >= 1:
                            emit_av(g - 1)
                    if qc >= 1:
                        emit_av(qc - 1)
                    emit_exp(qc)
                    emit_av(qc)
                    recip = apool.tile([P, QC], f32, tag="rc")
                    nc.vector.reciprocal(recip[:], sm[:])
                    y = apool.tile([P, QC], f16, tag=f"yn{hl}")
                    nc.vector.tensor_mul(y[:], yt[:, 0:QC], recip[:])
                    return y

                def oproj_part(b, qc, yn, quarters, os_sb):
                    # sub-interleaved so each quarter's PSUM drain overlaps
                    # the other sub's matmuls
                    for quarter in quarters:
                        for sub in range(2):
                            ops = opps.tile([P, 512], f32, tag="op", bufs=5)
                            for hl in range(2):
                                nc.tensor.matmul(
                                    ops[:],
                                    lhsT=yn[hl][:, sub * P : (sub + 1) * P],
                                    rhs=wo_sb[:, hl, quarter * 512 : (quarter + 1) * 512],
                                    start=(hl == 0),
                                    stop=(hl == 1),
                                )
                            if (quarter * 2 + sub) % 2 == 0:
                                nc.scalar.copy(
                                    os_sb[sub][:, quarter * 512 : (quarter + 1) * 512],
                                    ops[:],
                                )
                            else:
                                nc.vector.tensor_copy(
                                    os_sb[sub][:, quarter * 512 : (quarter + 1) * 512],
                                    ops[:],
                                )

                def oproj_alloc(b, qc):
                    return [
                        opool.tile([P, H], f16, tag="os", name=f"os{b}_{qc}_{s}")
                        for s in range(2)
                    ]

                def oproj_flush(b, qc, os_sb):
                    for sub in range(2):
                        t0 = qc * QC + sub * P
                        nc.sync.dma_start(out[b, t0 : t0 + P, :], os_sb[sub][:])

                pending = None

                def attn_step(b, qc):
                    # previous chunk's out-proj lands in two half-bursts
                    # around this chunk's second head-unit: PE work that
                    # hides the normalize chain and spreads evacuations
                    nonlocal pending
                    y0 = attn_unit(b, qc, 0)
                    if pending is not None:
                        pb, pqc, pyn = pending
                        osb = oproj_alloc(pb, pqc)
                        oproj_part(pb, pqc, pyn, (0, 1), osb)
                    y1 = attn_unit(b, qc, 1)
                    if pending is not None:
                        oproj_part(pb, pqc, pyn, (2, 3), osb)
                        oproj_flush(pb, pqc, osb)
                    pending = (b, qc, [y0, y1])

                # fully pipelined schedule: every attention unit runs in a
                # projection window as soon as causally possible, so the
                # exp/evac load always has proj matmuls to hide under and
                # there is no attention-only tail
                b0_units = {0: [], 1: [(0, 0)], 2: [(0, 1), (0, 2), (0, 3)],
                            3: [(0, 4), (0, 5)], 4: [(0, 6)]}
                b1_units = {0: [(0, 7), (1, 0), (1, 1)], 1: [(1, 2), (1, 3)],
                            2: [(1, 4), (1, 5)], 3: [(1, 6), (1, 7)]}
                for pi in range(len(chunks[0])):
                    proj_chunk(0, pi)
                    if pi + 1 < len(chunks[0]):
                        if pi + 1 >= 2:
                            prefetch_xt(0, pi + 1)
                    else:
                        prefetch_xt(1, 0)
                    rope_piece(0, pi)
                    for ub, uqc in b0_units[pi]:
                        attn_step(ub, uqc)
                for ci in range(len(chunks[1])):
                    proj_chunk(1, ci)
                    if ci + 1 < len(chunks[1]):
                        prefetch_xt(1, ci + 1)
                    rope_piece(1, ci)
                    for ub, uqc in b1_units[ci]:
                        attn_step(ub, uqc)
                pb, pqc, pyn = pending
                osb = oproj_alloc(pb, pqc)
                oproj_part(pb, pqc, pyn, (0, 1), osb)
                oproj_part(pb, pqc, pyn, (2, 3), osb)
                oproj_flush(pb, pqc, osb)
    nc.finalize()
    return nc


_NC_CACHE = None


def _get_program():
    global _NC_CACHE
    if _NC_CACHE is None:
        _NC_CACHE = _build_program()
    return _NC_CACHE


def _prep_in_maps(x, rotary, qkv_weight, o_weight):
    import jax
    import ml_dtypes

    bf = np.float16
    cpu = jax.devices("cpu")[0]
    with jax.default_device(cpu):
        import jax.numpy as jnp

        sq = jnp.mean(jnp.abs(jnp.asarray(qkv_weight)))
        wq_q = np.asarray(jnp.round(jnp.asarray(qkv_weight) / (sq + EPS)), np.float32)
        so = jnp.mean(jnp.abs(jnp.asarray(o_weight)))
        wo_q = np.asarray(jnp.round(jnp.asarray(o_weight) / (so + EPS)), np.float32)
        sq = float(sq)
        so = float(so)

    xt = np.ascontiguousarray(x.transpose(0, 2, 1)).astype(bf)
    cos_t = np.ascontiguousarray(rotary[1].T).astype(bf)
    sin_t = np.ascontiguousarray(rotary[0].T).astype(np.float32)
    sin_s = sin_t.copy()
    sin_s[:64] *= -1.0
    sin_s = sin_s.astype(bf)

    # aux: mask pair for the diagonal k-tile pair, identity, ones
    kk = np.arange(P)[:, None]
    qq = np.arange(QC)[None, :]
    aux = np.zeros((P, 832), np.float32)
    aux[:, 0:QC] = np.where(qq < kk, MASKV, 0.0)          # B0: k-tile 2qc
    aux[:, QC : 2 * QC] = np.where(qq < kk + P, MASKV, 0.0)  # B1: k-tile 2qc+1
    aux[:, 512:640] = np.eye(P)
    aux[:, 640:768] = 1.0
    aux[:, 768] = -8.0
    aux = aux.astype(bf)

    # fp16 scaling: sqrt(sm_scale) on BOTH q and k weights (scores land fully
    # scaled in PSUM, masks are in post-scale units); final o-scale folded
    # into v weights (keeps every fp16 tensor in normal range; o_weight stays
    # exactly ternary in fp16).
    alpha = np.float32(math.sqrt(sq * sq / math.sqrt(HEAD_DIM)))
    final_scale = np.float32(sq * so)

    in_maps = []
    for c in range(NCORES):
        rows = []
        for part in range(3):  # q, k, v blocks of qkv_weight
            for hl in range(HPC):
                g = HPC * c + hl
                blk = wq_q[part * H + g * HEAD_DIM : part * H + (g + 1) * HEAD_DIM]
                if part < 2:
                    blk = blk * alpha
                else:
                    blk = blk * final_scale
                rows.append(blk)
        wqkv_c = np.ascontiguousarray(np.concatenate(rows, axis=0).T).astype(bf)
        wo_c = np.ascontiguousarray(
            wo_q[:, c * FPC // 3 : (c + 1) * FPC // 3].T
        ).astype(bf)
        in_maps.append(
            {
                "xt": xt,
                "wqkv": wqkv_c,
                "wo": wo_c,
                "cos_t": cos_t,
                "sin_s": sin_s,
                "aux": aux,
            }
        )
    return in_maps


def kernel(x, rotary, qkv_weight, o_weight):
    from concourse.bass_utils import run_bass_kernel_spmd

    in_maps = _prep_in_maps(x, rotary, qkv_weight, o_weight)
    nc = _get_program()
    res = run_bass_kernel_spmd(nc, in_maps, core_ids=list(range(NCORES)))
    acc = res.results[0]["out"].astype(np.float32)
    for c in range(1, NCORES):
        acc = acc + res.results[c]["out"].astype(np.float32)
    return acc


# revision 65
# speedup vs baseline: 1.0340x; 1.0011x over previous
"""Megatron-style tensor-parallel causal attention (BitLinear qkv/o) on 8 TRN2 cores.

Sharding: each core owns 2 of 16 heads (qkv_weight rows) and the matching
256 o_weight columns. x/rotary replicated; partial outputs summed on host.

All SBUF data is f16 (halves DMA + enables DVE 2x modes); PSUM stays fp32.
Causal mask is folded into the score PSUM via an identity-lhsT matmul adding
-60 to masked entries before exp. Softmax denominator uses the all-ones
lhsT matmul (broadcast rows), normalization on DVE before the out-proj.
Emission order proj(b0), proj(b1), attn(b0), attn(b1) keeps the PE busy:
RoPE(b0) on DVE overlaps proj(b1) matmuls, attention overlaps nothing it
needs.
"""

import math

import numpy as np

EPS = 1e-5
NUM_HEADS = 16
HEAD_DIM = 128
B, S, H = 2, 2048, 2048
NCORES = 8
HPC = NUM_HEADS // NCORES        # heads per core = 2
FPC = 3 * HPC * HEAD_DIM         # qkv features per core = 768
P = 128
NHT = H // P                     # 16 h_in tiles
CH = 512                         # proj token chunk
NCH = S // CH                    # 4 chunks per batch
QC = 256                         # attention q chunk
NQC = S // QC                    # 8
MASKV = -60.0


def _build_program():
    import concourse.bacc as bacc
    import concourse.mybir as mybir
    import concourse.tile as tile

    f32 = mybir.dt.float32
    f16 = mybir.dt.float16
    AF = mybir.ActivationFunctionType

    nc = bacc.Bacc(None, target_bir_lowering=False)

    xt = nc.dram_tensor("xt", [B, H, S], f16, kind="ExternalInput")
    wqkv = nc.dram_tensor("wqkv", [H, FPC], f16, kind="ExternalInput")
    wo = nc.dram_tensor("wo", [HPC * HEAD_DIM, H], f16, kind="ExternalInput")
    cos_t = nc.dram_tensor("cos_t", [P, S], f16, kind="ExternalInput")
    sin_s = nc.dram_tensor("sin_s", [P, S], f16, kind="ExternalInput")
    # aux: [0:512) mask pair (B0|B1), [512:640) identity, [640:768) ones
    aux = nc.dram_tensor("aux", [P, 832], f16, kind="ExternalInput")
    out = nc.dram_tensor("out", [B, S, H], f16, kind="ExternalOutput")

    with tile.TileContext(nc) as tc:
        with tc.tile_pool(name="const", bufs=1) as cpool:
            # first proj chunk's x and the first weight slice lead the DMA
            # queue so the PE starts ~9us in instead of ~24us.
            w_sb = cpool.tile([P, NHT, FPC], f16)
            wre = wqkv.rearrange("(t p) f -> p t f", p=P)
            nc.sync.dma_start(w_sb[:, 0:4, :], wre[:, 0:4, :])

            with (
                tc.tile_pool(name="qk", bufs=2) as qkpool,
                tc.tile_pool(name="vv", bufs=2) as vpool,
                tc.tile_pool(name="work", bufs=2) as wpool,
                tc.tile_pool(name="attn", bufs=3) as apool,
                tc.tile_pool(name="outp", bufs=3) as opool,
                tc.psum_pool(name="pproj", bufs=2) as pps,
                tc.psum_pool(name="pop", bufs=2) as opps,
            ):
                # batch-0 leads with two small 256-token chunks so the first
                # proj matmuls start while weights are still streaming in
                chunks = {
                    0: [(0, 256), (256, 256), (512, 512), (1024, 512), (1536, 512)],
                    1: [(c * CH, CH) for c in range(NCH)],
                }
                # startup: interleave w and first-bite x by h-group
                xt0 = wpool.tile([P, NHT, 256], f16, tag="xt")
                xre0 = xt[0, :, 0:256].rearrange("(t p) c -> p t c", p=P)
                nc.sync.dma_start(xt0[:, 0:4, :], xre0[:, 0:4, :])
                for hgrp in range(1, 4):
                    nc.sync.dma_start(
                        w_sb[:, 4 * hgrp : 4 * (hgrp + 1), :],
                        wre[:, 4 * hgrp : 4 * (hgrp + 1), :],
                    )
                    nc.sync.dma_start(
                        xt0[:, 4 * hgrp : 4 * (hgrp + 1), :],
                        xre0[:, 4 * hgrp : 4 * (hgrp + 1), :],
                    )
                # second bite before the (later-needed) constants
                xt1 = wpool.tile([P, NHT, 256], f16, tag="xt")
                nc.sync.dma_start(
                    xt1[:], xt[0, :, 256:512].rearrange("(t p) c -> p t c", p=P)
                )
                wo_sb = cpool.tile([P, HPC, H], f16)
                nc.sync.dma_start(wo_sb[:], wo.rearrange("(t p) o -> p t o", p=P))
                aux_sb = cpool.tile([P, 832], f16)
                nc.sync.dma_start(aux_sb[:], aux[:])
                rot_sb = cpool.tile([P, 2 * S], f16)
                nc.sync.dma_start(rot_sb[:, 0:S], cos_t[:])
                nc.sync.dma_start(rot_sb[:, S : 2 * S], sin_s[:])

                msk = aux_sb[:, 0:512]          # [k,128] x (B0|B1) for diag pair
                iden = aux_sb[:, 512:640]       # identity
                ones = aux_sb[:, 640:768]       # all-ones
                expb = aux_sb[:, 768:769]       # exp bias column (-8)

                qk_raw = {}   # (b, f) -> raw (pre-rope) tiles
                qk_rope = {}  # (b, f) -> roped tiles
                v_sb = {}     # b -> v tiles [tok_part, ktile, hl*128]
                for b in range(B):
                    for f in range(4):
                        qk_raw[b, f] = qkpool.tile(
                            [P, S], f16, tag=f"qkr{f}", name=f"qkr{f}_{b}"
                        )
                        qk_rope[b, f] = qkpool.tile(
                            [P, S], f16, tag=f"qkf{f}", name=f"qkf{f}_{b}"
                        )
                    v_sb[b] = vpool.tile(
                        [P, (S // P) * 2 * P], f16, tag="v", name=f"v_{b}"
                    )

                # ---------------- projection (+rope) -----------------------
                xt_pre = {}

                def prefetch_xt(b, ci):
                    t0c, W = chunks[b][ci]
                    t = wpool.tile(
                        [P, NHT, W], f16, tag="xt", name=f"xt_{b}_{ci}"
                    )
                    nc.sync.dma_start(
                        t[:],
                        xt[b, :, t0c : t0c + W].rearrange("(t p) c -> p t c", p=P),
                    )
                    xt_pre[b, ci] = t

                def proj_chunk(b, ci):
                    t0c, W = chunks[b][ci]
                    if True:
                        if b == 0 and ci == 0:
                            xt_sb = xt0
                        elif b == 0 and ci == 1:
                            xt_sb = xt1
                        elif (b, ci) in xt_pre:
                            xt_sb = xt_pre.pop((b, ci))
                        else:
                            xt_sb = wpool.tile(
                                [P, NHT, W], f16,
                                tag="xt", name=f"xt_{b}_{ci}",
                            )
                            nc.sync.dma_start(
                                xt_sb[:],
                                xt[b, :, t0c : t0c + W].rearrange(
                                    "(t p) c -> p t c", p=P
                                ),
                            )
                        # q0,q1,k0,k1 : [feat, tok]
                        for f in range(4):
                            ps = pps.tile([P, W], f32, tag="proj", name=f"ps{b}_{ci}_{f}")
                            for h in range(NHT):
                                nc.tensor.matmul(
                                    ps[:],
                                    lhsT=w_sb[:, h, f * P : (f + 1) * P],
                                    rhs=xt_sb[:, h, :],
                                    start=(h == 0),
                                    stop=(h == NHT - 1),
                                )
                            if f % 2 == 0:
                                nc.scalar.copy(
                                    qk_raw[b, f][:, t0c : t0c + W], ps[:]
                                )
                            else:
                                nc.vector.tensor_copy(
                                    qk_raw[b, f][:, t0c : t0c + W], ps[:]
                                )
                        # v: [tok, feat] two tok-subs per psum tile
                        for half in range(W // 256):
                            psv = pps.tile(
                                [P, 512], f32, tag="proj", name=f"psv{b}_{ci}_{half}"
                            )
                            for sub in range(2):
                                tsub = half * 2 + sub
                                for h in range(NHT):
                                    nc.tensor.matmul(
                                        psv[:, sub * 2 * P : (sub + 1) * 2 * P],
                                        lhsT=xt_sb[:, h, tsub * P : (tsub + 1) * P],
                                        rhs=w_sb[:, h, 4 * P : 6 * P],
                                        start=(h == 0),
                                        stop=(h == NHT - 1),
                                    )
                            kt0 = t0c // P + half * 2
                            if half == 0:
                                nc.scalar.copy(
                                    v_sb[b][:, kt0 * 2 * P : (kt0 + 2) * 2 * P], psv[:]
                                )
                            else:
                                nc.vector.tensor_copy(
                                    v_sb[b][:, kt0 * 2 * P : (kt0 + 2) * 2 * P], psv[:]
                                )
                def rope_piece(b, pi):
                    # rope one proj chunk's span; runs on DVE under the next
                    # chunk's proj matmuls
                    t0c, W = chunks[b][pi]
                    for f in range(4):
                        raw = qk_raw[b, f]
                        qsw = wpool.tile(
                            [P, W], f16, tag="qsw", name=f"qsw{b}_{pi}_{f}"
                        )
                        nc.sync.dma_start(
                            qsw[0:64, :], raw[64:128, t0c : t0c + W]
                        )
                        nc.sync.dma_start(
                            qsw[64:128, :], raw[0:64, t0c : t0c + W]
                        )
                        m1 = wpool.tile(
                            [P, W], f16, tag="m1", name=f"m1{b}_{pi}_{f}"
                        )
                        nc.vector.tensor_mul(
                            m1[:], raw[:, t0c : t0c + W], rot_sb[:, t0c : t0c + W]
                        )
                        nc.vector.tensor_mul(
                            qsw[:], qsw[:], rot_sb[:, S + t0c : S + t0c + W]
                        )
                        nc.vector.tensor_add(
                            qk_rope[b, f][:, t0c : t0c + W], m1[:], qsw[:]
                        )

                # ---------------- attention + out-proj ----------------------
                # The last k-tile of each q-chunk only covers q[128:256)
                # (ragged trim). Denominator: full pairs are pre-summed on DVE
                # (halves the ones-matmul rows); the ones-matmul for pair g is
                # deferred until after pair g+1's attn*v so the PE never waits
                # on the DVE add.
                def attn_unit(b, qc, hl):
                    q_t = qk_rope[b, hl]
                    k_t = qk_rope[b, 2 + hl]
                    qs = q_t[:, qc * QC : (qc + 1) * QC]
                    qs_hi = q_t[:, qc * QC + P : (qc + 1) * QC]
                    yt = opps.tile([P, 512], f32, tag="op", name=f"yt{b}_{qc}_{hl}", bufs=5)
                    sm = opps.tile([P, QC], f32, tag="sum", name=f"sm{b}_{qc}_{hl}", bufs=1)
                    pend = None       # deferred exs tile for the ones-matmul
                    sum_started = False

                    def ones_mm(rhs_ap, region, stop):
                        nonlocal sum_started
                        nc.tensor.matmul(
                            sm[:, region[0] : region[1]],
                            lhsT=ones,
                            rhs=rhs_ap,
                            start=not sum_started,
                            stop=stop,
                        )
                        sum_started = True

                    def emit_scores(g):
                        diag = g == qc
                        sc = opps.tile(
                            [P, 2 * QC], f32, tag="op", bufs=5,
                            name=f"sc{b}_{qc}_{hl}_{g}",
                        )
                        nc.tensor.matmul(
                            sc[:, 0:QC],
                            lhsT=k_t[:, 2 * g * P : (2 * g + 1) * P],
                            rhs=qs,
                            start=True,
                            stop=not diag,
                        )
                        if diag:
                            # only the left [128,128] of this tile is masked
                            nc.tensor.matmul(
                                sc[:, 0:P], lhsT=iden, rhs=msk[:, 0:P],
                                start=False, stop=True,
                            )
                            nc.tensor.matmul(
                                sc[:, QC : QC + P],
                                lhsT=k_t[:, (2 * g + 1) * P : (2 * g + 2) * P],
                                rhs=qs_hi,
                                start=True,
                                stop=False,
                            )
                            nc.tensor.matmul(
                                sc[:, QC : QC + P], lhsT=iden, rhs=msk[:, 0:P],
                                start=False, stop=True,
                            )
                        else:
                            nc.tensor.matmul(
                                sc[:, QC : 2 * QC],
                                lhsT=k_t[:, (2 * g + 1) * P : (2 * g + 2) * P],
                                rhs=qs,
                                start=True,
                                stop=True,
                            )
                        return sc

                    # 3-stage pipeline: scores(g+2) and exp(g+1) run ahead of
                    # attn*v(g), so the PE never waits on the Activation
                    # engine's exp. Denominator adds (DVE) get a full
                    # iteration of slack before their ones-matmul.
                    exd = {}   # g -> (ex tile, exs tile or None)

                    def emit_exp(g):
                        nonlocal qpend, ppend
                        diag = g == qc
                        scw = 2 * QC if not diag else QC + P
                        ex = apool.tile([P, scw], f16, tag="ex")
                        nc.scalar.activation(
                            ex[:], scd[g][:, 0:scw], AF.Exp, bias=expb
                        )
                        if not diag:
                            exs = apool.tile([P, QC], f16, tag="exs", bufs=4)
                            nc.vector.tensor_add(
                                exs[:], ex[:, 0:QC], ex[:, QC : 2 * QC]
                            )
                        else:
                            # fold the whole diagonal into a leftover pending
                            # sum when one exists (its ones-matmuls vanish);
                            # otherwise keep the two 128-row matmul form
                            tgt = ppend if ppend is not None else qpend
                            if tgt is not None:
                                nc.vector.tensor_add(
                                    tgt[:, 0:P], tgt[:, 0:P], ex[:, 0:P]
                                )
                                nc.vector.tensor_add(
                                    tgt[:, P:QC], tgt[:, P:QC], ex[:, P:QC]
                                )
                                nc.vector.tensor_add(
                                    tgt[:, P:QC], tgt[:, P:QC], ex[:, QC : QC + P]
                                )
                                dmerged[0] = True
                                exs = None
                            else:
                                exs = apool.tile([P, P], f16, tag="exs", bufs=4)
                                nc.vector.tensor_add(
                                    exs[:], ex[:, P:QC], ex[:, QC : QC + P]
                                )
                        exd[g] = (ex, exs)

                    def emit_av(g):
                        nonlocal qpend, ppend, opend
                        diag = g == qc
                        scw = 2 * QC if not diag else QC + P
                        ex, exs = exd.pop(g)
                        v0 = 2 * g * 2 * P + hl * P
                        nc.tensor.matmul(
                            yt[:, 0:QC],
                            lhsT=v_sb[b][:, v0 : v0 + P],
                            rhs=ex[:, 0:QC],
                            start=(g == 0),
                            stop=False,
                        )
                        v1 = (2 * g + 1) * 2 * P + hl * P
                        nc.tensor.matmul(
                            yt[:, P:QC] if diag else yt[:, 0:QC],
                            lhsT=v_sb[b][:, v1 : v1 + P],
                            rhs=ex[:, QC:scw],
                            start=False,
                            stop=diag,
                        )
                        # quad-summed denominator: ones-matmuls run on
                        # pair-of-pair sums, each deferred one iteration so
                        # the PE never waits on the DVE adds
                        if opend is not None:
                            ones_mm(opend[:], (0, QC), stop=False)
                            opend = None
                        if not diag:
                            if qpend is None:
                                qpend = exs
                            else:
                                exq = apool.tile(
                                    [P, QC], f16, tag="exq",
                                    name=f"exq{b}_{qc}_{hl}_{g}", bufs=5,
                                )
                                nc.vector.tensor_add(exq[:], qpend[:], exs[:])
                                qpend = None
                                if ppend is None:
                                    ppend = exq
                                else:
                                    exo = apool.tile(
                                        [P, QC], f16, tag="exo",
                                        name=f"exo{b}_{qc}_{hl}_{g}",
                                    )
                                    nc.vector.tensor_add(
                                        exo[:], ppend[:], exq[:]
                                    )
                                    ppend = None
                                    opend = exo
                        elif qc == 0:
                            # no prior pair zeroed the region: cover all of it
                            ones_mm(ex[:, 0:QC], (0, QC), stop=False)
                            ones_mm(ex[:, QC : QC + P], (P, QC), stop=True)
                        elif dmerged[0]:
                            flushes = [t for t in (ppend, qpend) if t is not None]
                            for i, t in enumerate(flushes):
                                ones_mm(
                                    t[:], (0, QC), stop=(i == len(flushes) - 1)
                                )
                            ppend = qpend = None
                        else:
                            if ppend is not None:
                                ones_mm(ppend[:], (0, QC), stop=False)
                                ppend = None
                            if qpend is not None:
                                ones_mm(qpend[:], (0, QC), stop=False)
                                qpend = None
                            ones_mm(ex[:, 0:P], (0, P), stop=False)
                            ones_mm(exs[:], (P, QC), stop=True)

                    dmerged = [False]  # diag piece folded into a pending sum
                    qpend = None   # pair sum awaiting its quad partner
                    ppend = None   # quad sum awaiting its octet partner
                    opend = None   # tree sum awaiting its ones-matmul
                    scd = {0: emit_scores(0)}
                    if qc >= 1:
                        scd[1] = emit_scores(1)
                    for g in range(qc):
                        emit_exp(g)
                        if g + 2 <= qc:
                            scd[g + 2] = emit_scores(g + 2)
                        if g >= 1:
                            emit_av(g - 1)
                    if qc >= 1:
                        emit_av(qc - 1)
                    emit_exp(qc)
                    emit_av(qc)
                    recip = apool.tile([P, QC], f32, tag="rc")
                    nc.vector.reciprocal(recip[:], sm[:])
                    y = apool.tile([P, QC], f16, tag=f"yn{hl}")
                    nc.vector.tensor_mul(y[:], yt[:, 0:QC], recip[:])
                    return y

                def oproj_part(b, qc, yn, quarters, os_sb):
                    # sub-interleaved so each quarter's PSUM drain overlaps
                    # the other sub's matmuls
                    for quarter in quarters:
                        for sub in range(2):
                            ops = opps.tile([P, 512], f32, tag="op", bufs=5)
                            for hl in range(2):
                                nc.tensor.matmul(
                                    ops[:],
                                    lhsT=yn[hl][:, sub * P : (sub + 1) * P],
                                    rhs=wo_sb[:, hl, quarter * 512 : (quarter + 1) * 512],
                                    start=(hl == 0),
                                    stop=(hl == 1),
                                )
                            if (quarter * 2 + sub) % 2 == 0:
                                nc.scalar.copy(
                                    os_sb[sub][:, quarter * 512 : (quarter + 1) * 512],
                                    ops[:],
                                )
                            else:
                                nc.vector.tensor_copy(
                                    os_sb[sub][:, quarter * 512 : (quarter + 1) * 512],
                                    ops[:],
                                )

                def oproj_alloc(b, qc):
                    return [
                        opool.tile([P, H], f16, tag="os", name=f"os{b}_{qc}_{s}")
                        for s in range(2)
                    ]

                def oproj_flush(b, qc, os_sb):
                    for sub in range(2):
                        t0 = qc * QC + sub * P
                        nc.sync.dma_start(out[b, t0 : t0 + P, :], os_sb[sub][:])

                pending = None

                def attn_step(b, qc):
                    # previous chunk's out-proj lands in two half-bursts
                    # around this chunk's second head-unit: PE work that
                    # hides the normalize chain and spreads evacuations
                    nonlocal pending
                    y0 = attn_unit(b, qc, 0)
                    if pending is not None:
                        pb, pqc, pyn = pending
                        osb = oproj_alloc(pb, pqc)
                        oproj_part(pb, pqc, pyn, (0, 1), osb)
                    y1 = attn_unit(b, qc, 1)
                    if pending is not None:
                        oproj_part(pb, pqc, pyn, (2, 3), osb)
                        oproj_flush(pb, pqc, osb)
                    pending = (b, qc, [y0, y1])

                # fully pipelined schedule: every attention unit runs in a
                # projection window as soon as causally possible, so the
                # exp/evac load always has proj matmuls to hide under and
                # there is no attention-only tail
                b0_units = {0: [], 1: [(0, 0)], 2: [(0, 1), (0, 2), (0, 3)],
                            3: [(0, 4), (0, 5)], 4: [(0, 6)]}
                b1_units = {0: [(0, 7), (1, 0), (1, 1)], 1: [(1, 2), (1, 3)],
                            2: [(1, 4), (1, 5)], 3: [(1, 6), (1, 7)]}
                for pi in range(len(chunks[0])):
                    proj_chunk(0, pi)
                    if pi + 1 < len(chunks[0]):
                        if pi + 1 >= 2:
                            prefetch_xt(0, pi + 1)
                    else:
                        prefetch_xt(1, 0)
                    rope_piece(0, pi)
                    for ub, uqc in b0_units[pi]:
                        attn_step(ub, uqc)
                for ci in range(len(chunks[1])):
                    proj_chunk(1, ci)
                    if ci + 1 < len(chunks[1]):
                        prefetch_xt(1, ci + 1)
                    rope_piece(1, ci)
                    for ub, uqc in b1_units[ci]:
                        attn_step(ub, uqc)
                pb, pqc, pyn = pending
                osb = oproj_alloc(pb, pqc)
                oproj_part(pb, pqc, pyn, (0, 1), osb)
                oproj_part(pb, pqc, pyn, (2, 3), osb)
                oproj_flush(pb, pqc, osb)
    nc.finalize()
    return nc


_NC_CACHE = None


def _get_program():
    global _NC_CACHE
    if _NC_CACHE is None:
        _NC_CACHE = _build_program()
    return _NC_CACHE


def _prep_in_maps(x, rotary, qkv_weight, o_weight):
    import jax
    import ml_dtypes

    bf = np.float16
    cpu = jax.devices("cpu")[0]
    with jax.default_device(cpu):
        import jax.numpy as jnp

        sq = jnp.mean(jnp.abs(jnp.asarray(qkv_weight)))
        wq_q = np.asarray(jnp.round(jnp.asarray(qkv_weight) / (sq + EPS)), np.float32)
        so = jnp.mean(jnp.abs(jnp.asarray(o_weight)))
        wo_q = np.asarray(jnp.round(jnp.asarray(o_weight) / (so + EPS)), np.float32)
        sq = float(sq)
        so = float(so)

    xt = np.ascontiguousarray(x.transpose(0, 2, 1)).astype(bf)
    cos_t = np.ascontiguousarray(rotary[1].T).astype(bf)
    sin_t = np.ascontiguousarray(rotary[0].T).astype(np.float32)
    sin_s = sin_t.copy()
    sin_s[:64] *= -1.0
    sin_s = sin_s.astype(bf)

    # aux: mask pair for the diagonal k-tile pair, identity, ones
    kk = np.arange(P)[:, None]
    qq = np.arange(QC)[None, :]
    aux = np.zeros((P, 832), np.float32)
    aux[:, 0:QC] = np.where(qq < kk, MASKV, 0.0)          # B0: k-tile 2qc
    aux[:, QC : 2 * QC] = np.where(qq < kk + P, MASKV, 0.0)  # B1: k-tile 2qc+1
    aux[:, 512:640] = np.eye(P)
    aux[:, 640:768] = 1.0
    aux[:, 768] = -8.0
    aux = aux.astype(bf)

    # fp16 scaling: sqrt(sm_scale) on BOTH q and k weights (scores land fully
    # scaled in PSUM, masks are in post-scale units); final o-scale folded
    # into v weights (keeps every fp16 tensor in normal range; o_weight stays
    # exactly ternary in fp16).
    alpha = np.float32(math.sqrt(sq * sq / math.sqrt(HEAD_DIM)))
    final_scale = np.float32(sq * so)

    in_maps = []
    for c in range(NCORES):
        rows = []
        for part in range(3):  # q, k, v blocks of qkv_weight
            for hl in range(HPC):
                g = HPC * c + hl
                blk = wq_q[part * H + g * HEAD_DIM : part * H + (g + 1) * HEAD_DIM]
                if part < 2:
                    blk = blk * alpha
                else:
                    blk = blk * final_scale
                rows.append(blk)
        wqkv_c = np.ascontiguousarray(np.concatenate(rows, axis=0).T).astype(bf)
        wo_c = np.ascontiguousarray(
            wo_q[:, c * FPC // 3 : (c + 1) * FPC // 3].T
        ).astype(bf)
        in_maps.append(
            {
                "xt": xt,
                "wqkv": wqkv_c,
                "wo": wo_c,
                "cos_t": cos_t,
                "sin_s": sin_s,
                "aux": aux,
            }
        )
    return in_maps


def kernel(x, rotary, qkv_weight, o_weight):
    from concourse.bass_utils import run_bass_kernel_spmd

    in_maps = _prep_in_maps(x, rotary, qkv_weight, o_weight)
    nc = _get_program()
    res = run_bass_kernel_spmd(nc, in_maps, core_ids=list(range(NCORES)))
    acc = res.results[0]["out"].astype(np.float32)
    for c in range(1, NCORES):
        acc = acc + res.results[c]["out"].astype(np.float32)
    return acc
